# revision 39
# baseline (speedup 1.0000x reference)
"""CrossFusionBlock Trainium2 kernel.

Dual-stream cross-attention block (B=8, C=512, HW=1024, 8 heads, FFN 2048).
Sharding: data-parallel over batch across 8 NeuronCores (1 batch element per
core), weights replicated. All weight transposes / casts / permutations are
done on the host so the device kernel contains no transposes at all.

v2: ACT (scalar engine) is the bottleneck (softmax exp 131k lanes-cycles +
gelu). Everything else is scheduled around keeping ACT saturated:
  - all projections fp8 DoubleRow (matmul cost = out-cols only, fp8 DR
    halves it); proj weights host-scaled x16 so fp8 e4m3 is well covered.
    Attention product scale 16*16=256 absorbed by LN1 (x256 residual).
  - p-major host layouts: every weight/activation load is one contiguous
    DMA; params ride in a single [P,2,48] block. First exp at ~5us.
  - softmax normalize: reciprocal writes partition 0 (misaligned DVE op),
    Pool partition_broadcast fans out 1/Z, both head-halves written by DVE
    tensor ops (par1 with misaligned out partitions) -- no DMAs at all.
  - LN mu/rs broadcasts on Pool partition_broadcast instead of DMA.
  - ACT order: [exps | LN1(s) rsqrt inline (shared Ln+Exp table set),
    gelus(s), LN1(f) rsqrt, gelus(f), LN2(s+f) rsqrt] -> 4 table reloads,
    no ACT stalls. LN1(f) stats emitted early, its ACT part late.
  - FFN2/LN2 chunk-interleaved so the tail chain is one half-stream deep.
"""

import sys

import numpy as np

for _p in ("/opt/trn_rl_repo", "/opt/pypackages"):
    if _p not in sys.path:
        sys.path.insert(0, _p)

import ml_dtypes  # noqa: E402

import concourse.bass as bass  # noqa: E402
from concourse import bacc  # noqa: E402
import concourse.mybir as mybir  # noqa: E402
import concourse.tile as tile  # noqa: E402


def _patch_act_tables():
    """Make natural_log_exp_and_others the only set offering Exp/Ln.

    The table-load pass greedily picks the first set containing each
    activation function, which ping-pongs between the ln-only and exp-only
    sets (2 table loads per LayerNorm rsqrt). Hiding Exp/Ln from the other
    sets forces the combined set; set ids keep their true act_info indices
    so the emitted LoadActFuncSet ids stay valid for walrus.
    """
    import concourse.hw_specs as hw_specs

    if getattr(hw_specs, "_act_tables_patched", False):
        return
    orig = hw_specs.get_activation_tables

    def patched(arch):
        tabs = dict(orig(arch))
        exp = mybir.ActivationFunctionType.Exp
        ln = mybir.ActivationFunctionType.Ln
        out = {}
        for name, funcs in tabs.items():
            if name != "natural_log_exp_and_others":
                funcs = funcs - {exp, ln}
            out[name] = funcs
        return out

    hw_specs._act_tables_patched = True
    hw_specs.get_activation_tables = patched
    bacc.get_activation_tables = patched


_patch_act_tables()

P = 128
C = 512
HW = 1024
HEADS = 8
DH = 64
HID = 2048
CT = C // P        # 4 channel tiles
HT = HID // P      # 16 hidden tiles
TT = HW // P       # 8 token tiles
NCH = HW // 512    # 2 free-dim chunks of 512
EPS = 1e-6
BF16 = mybir.dt.bfloat16
FP8 = mybir.dt.float8e4
F32 = mybir.dt.float32
AF = mybir.ActivationFunctionType
ALU = mybir.AluOpType
DR = mybir.MatmulPerfMode.DoubleRow

N_CORES = 8
B, H_IMG, W_IMG = 8, 32, 32

VW = 72  # V row width: DH + ones col + zero pad (16B-aligned for DoubleRow)

# param block column layout: 8 x [P, CT] then b1 [P, HT]
PNAMES = ("bo256", "n1w", "n1b", "n1w16", "n1b16", "n2w", "n2b", "b216")
NPCOL = len(PNAMES) * CT + HT  # 48


# --------------------------------------------------------------------------
# device program
# --------------------------------------------------------------------------

def _emit_proj_qk_one(tc, pools, x8, w8, out2, ts=tuple(range(CT)),
                      act_evict=False):
    """fp8 DR projection with head-grouped permuted W -> out2 = [g0, g1].

    Psum tile t = 2*g + hi holds rows (head 4g+h4, d = 32*hi + lo) at
    partition 32*h4 + lo; evacuated to out2[g][:, hi, :]. act_evict
    alternates psum eviction between ACT and DVE -- only useful in the
    head, while ACT still has no exps to chew on.
    """
    nc = tc.nc
    psum_mm = pools["psum_mm"]
    i = 0
    for t in ts:
        g, hi = t // 2, t % 2
        for ch in range(NCH):
            pq = psum_mm.tile([P, 512], F32, tag="mm", name="mm")
            for k in range(CT // 2):
                nc.tensor.matmul(
                    pq,
                    lhsT=w8[:, 2 * k:2 * k + 2, t * P:(t + 1) * P],
                    rhs=x8[:, 2 * k:2 * k + 2, ch * 512:(ch + 1) * 512],
                    start=(k == 0), stop=(k == CT // 2 - 1),
                    perf_mode=DR,
                )
            dst = out2[g][:, hi, ch * 512:(ch + 1) * 512]
            if act_evict and i % 2 == 0:
                nc.scalar.copy(out=dst, in_=pq)
            else:
                nc.vector.tensor_copy(out=dst, in_=pq)
            i += 1


def _emit_proj_v(tc, pools, xkv8, wv8, v_hf):
    nc = tc.nc
    psum_mm = pools["psum_mm"]
    for tt in range(TT):
        pv = psum_mm.tile([P, 512], F32, tag="mm", name="mm")
        for k in range(CT // 2):
            nc.tensor.matmul(
                pv,
                lhsT=xkv8[:, 2 * k:2 * k + 2, tt * P:(tt + 1) * P],
                rhs=wv8[:, 2 * k:2 * k + 2, :],
                start=(k == 0), stop=(k == CT // 2 - 1),
                perf_mode=DR,
            )
        nc.vector.tensor_copy(
            out=v_hf[:, tt, :, 0:DH],
            in_=pv.rearrange("p (h d) -> p h d", d=DH),
        )
        nc.vector.memset(v_hf[:, tt, :, DH:DH + 1], 1.0)


def _emit_st_exp(tc, pools, hp, q2g, k2g, filler=None):
    """S^T per head via fp8 DoubleRow (Ki=32 x2) -> exp(P^T*2^-11) fp8.

    filler() is emitted after tt==1: its PE work rides in the huge PE
    slack inside the st block instead of BETWEEN st blocks, where it
    would delay the next block's S^T matmuls and starve the exp stream.
    """
    nc = tc.nc
    pt = {}
    for par in (0, 1):
        pt[par] = pools["pt"].tile([P, TT, HW], FP8, tag="pt", name="pt",
                                   bufs=4)
    for tt in range(TT):
        if tt == 2 and filler is not None:
            filler()
        ps = {}
        for par in (0, 1):
            h4 = (2 * hp + par) % 4
            base = 32 * h4
            kw = {"tile_position": (96, 0)} if h4 == 3 else {}
            p_s = pools["psum_s"].tile([P, HW], F32, tag="s", name="s")
            for ch in range(NCH):
                nc.tensor.matmul(
                    p_s[:, ch * 512:(ch + 1) * 512],
                    lhsT=k2g[base:base + 32, :, tt * P:(tt + 1) * P],
                    rhs=q2g[base:base + 32, :, ch * 512:(ch + 1) * 512],
                    start=True, stop=True,
                    perf_mode=DR,
                    **kw,
                )
            ps[par] = p_s
        for par in (0, 1):
            nc.scalar.activation(out=pt[par][:, tt, :], in_=ps[par],
                                 func=AF.Exp, scale=0.125 / 256.0)
    return pt


def _emit_av(tc, pools, hp, pt, v_hf, o_pair):
    """AV+Z (ones column) fp8 DR -> normalize into o_pair[:, hp].

    Z sits at psum row DH; its reciprocal is written to partition 0 of a
    [1,512] tile (misaligned DVE op), Pool broadcasts it to 64 partitions,
    then both head-halves are normalized by DVE tensor ops (par1 writes
    partitions 64:128 from psum rows 0:64 -- misaligned out).
    """
    nc = tc.nc
    for par in (0, 1):
        h = 2 * hp + par
        for ch in range(NCH):
            sl = slice(ch * 512, (ch + 1) * 512)
            pav = pools["psum_av"].tile([P, 512], F32, tag="av", name="av")
            for tt2 in range(TT // 2):
                nc.tensor.matmul(
                    pav[0:VW, :],
                    lhsT=v_hf[:, 2 * tt2:2 * tt2 + 2, h, :],
                    rhs=pt[par][:, 2 * tt2:2 * tt2 + 2, sl],
                    start=(tt2 == 0), stop=(tt2 == TT // 2 - 1),
                    perf_mode=DR,
                )
            rz0 = pools["rz"].tile([1, 512], BF16, tag="rz0", name="rz0",
                                   bufs=4)
            with nc.allow_low_precision(reason="1/Z feeds an fp8 store"):
                nc.vector.reciprocal(out=rz0, in_=pav[DH:DH + 1, :])
            rzb = pools["rz"].tile([DH, 512], BF16, tag="rzb", name="rzb",
                                   bufs=4)
            nc.gpsimd.partition_broadcast(rzb, rz0, channels=DH)
            nc.vector.tensor_tensor(
                o_pair[par * DH:(par + 1) * DH, hp, sl],
                pav[0:DH, :], rzb, ALU.mult,
            )


def _emit_wo_residual(tc, pools, pfx, io, cts):
    """Wo(16x fp8) projection + 256*bo + 256*x residual -> r_bf (=256*r)."""
    nc = tc.nc
    o_hf = io["o"]
    wo, params = io["wo"], io["params"]
    psum_mm = pools["psum_mm"]
    if "r" not in io:
        io["r"] = pools["r_pool"].tile([P, CT, HW], BF16, tag=f"r_{pfx}",
                                       name=f"r_{pfx}")
    r_bf = io["r"]
    for ct in cts:
        x256 = pools["xr"].tile([P, HW], BF16, tag="xr", name="xr", bufs=2)
        nc.gpsimd.dma_start(out=x256, in_=io["x256d"][:, ct, :])
        for ch in range(NCH):
            sl = slice(ch * 512, (ch + 1) * 512)
            pe_ = psum_mm.tile([P, 512], F32, tag="mm", name="mm")
            for i2 in range(HEADS // 4):
                nc.tensor.matmul(
                    pe_,
                    lhsT=wo[:, 2 * i2:2 * i2 + 2, ct * P:(ct + 1) * P],
                    rhs=o_hf[:, 2 * i2:2 * i2 + 2, sl],
                    start=(i2 == 0), stop=(i2 == HEADS // 4 - 1),
                    perf_mode=DR,
                )
            nc.vector.scalar_tensor_tensor(
                out=r_bf[:, ct, sl], in0=pe_,
                scalar=params["bo256"][:, ct:ct + 1],
                in1=x256[:, sl], op0=ALU.add, op1=ALU.add,
            )


def _emit_ln_stats(tc, pools, src_bf, st, inv512, eps_sb,
                   chunks=tuple(range(NCH)), sq_dve=False, rsqrt=True):
    """LN stats over the channel (partition x 4-tile) axis of [P, CT, HW].

    Fills st["mu2"]/st["rs2"] rows ([1, NCH, 512]). rsqrt=True also emits
    the ACT Ln+Exp pair per chunk; rsqrt=False defers it to
    _emit_ln_rsqrt (so the ACT ops can be queued later).
    """
    nc = tc.nc
    psum_mm = pools["psum_mm"]
    if "mu2" not in st:
        st["mu2"] = pools["rows"].tile([1, NCH, 512], BF16, tag="mu2",
                                       name="mu2", bufs=2)
        st["var2"] = pools["rows"].tile([1, NCH, 512], F32, tag="var2",
                                        name="var2", bufs=2)
        st["rs2"] = pools["rows"].tile([1, NCH, 512], BF16, tag="rs2",
                                       name="rs2", bufs=2)
    mu2, var2, rs2 = st["mu2"], st["var2"], st["rs2"]
    for ch in chunks:
        sl = slice(ch * 512, (ch + 1) * 512)
        pmu = psum_mm.tile([1, 512], F32, tag="mm", name="mm")
        for k in range(CT):
            nc.tensor.matmul(
                pmu, lhsT=inv512[:, 0:1], rhs=src_bf[:, k, sl],
                start=(k == 0), stop=(k == CT - 1),
            )
        pms = psum_mm.tile([1, 512], F32, tag="mm", name="mm")
        for k in range(CT):
            r2 = pools["sq"].tile([P, 512], BF16, tag="sq", name="sq")
            if sq_dve == "alt":
                sq_eng = nc.vector if k % 2 else nc.gpsimd
            else:
                sq_eng = nc.vector if sq_dve else nc.gpsimd
            sq_eng.tensor_tensor(r2, src_bf[:, k, sl], src_bf[:, k, sl],
                                 ALU.mult)
            nc.tensor.matmul(
                pms, lhsT=inv512[:, 0:1], rhs=r2,
                start=(k == 0), stop=(k == CT - 1),
            )
        nc.vector.tensor_copy(out=mu2[0:1, ch, :], in_=pmu)
        musq = pools["rows"].tile([1, 512], F32, tag="musq", name="musq",
                                  bufs=2)
        nc.vector.tensor_tensor(musq, mu2[0:1, ch, :], mu2[0:1, ch, :],
                                ALU.mult)
        nc.vector.tensor_tensor(var2[0:1, ch, :], pms, musq, ALU.subtract)
        if rsqrt:
            _emit_ln_rsqrt(tc, pools, st, eps_sb, chunks=(ch,))


def _emit_ln_rsqrt(tc, pools, st, eps_sb, chunks=tuple(range(NCH))):
    """rs = 1/sqrt(var+eps) = exp(-0.5*ln(var+eps)) on ACT (Ln+Exp set)."""
    nc = tc.nc
    for ch in chunks:
        lnv = pools["rows"].tile([1, 512], F32, tag="lnv", name="lnv", bufs=2)
        nc.scalar.activation(lnv, st["var2"][0:1, ch, :], AF.Ln,
                             bias=eps_sb[:, 0:1])
        nc.scalar.activation(st["rs2"][0:1, ch, :], lnv, AF.Exp, scale=-0.5)


def _emit_ln_norm(tc, pools, src_bf, st, out_writer,
                  chunks=tuple(range(NCH)), pool_cts=()):
    """Broadcast mu/rs (Pool) and hand normalized [P,512] bf16 pieces on.

    pool_cts: ct indices whose subtract/mult pair runs on Pool instead of
    DVE (load-balancing for windows where DVE is saturated).
    """
    nc = tc.nc
    for ch in chunks:
        sl = slice(ch * 512, (ch + 1) * 512)
        mu_b = pools["bcast"].tile([P, 512], BF16, tag="mu_b", name="mu_b",
                                   bufs=2)
        rs_b = pools["bcast"].tile([P, 512], BF16, tag="rs_b", name="rs_b",
                                   bufs=2)
        nc.gpsimd.partition_broadcast(mu_b, st["mu2"][0:1, ch, :], channels=P)
        nc.gpsimd.partition_broadcast(rs_b, st["rs2"][0:1, ch, :], channels=P)
        for ct in range(CT):
            eng = nc.gpsimd if ct in pool_cts else nc.vector
            tmp = pools["tmp"].tile([P, 512], BF16, tag="tmp", name="tmp",
                                    bufs=2)
            eng.tensor_tensor(tmp, src_bf[:, ct, sl], mu_b, ALU.subtract)
            eng.tensor_tensor(tmp, tmp, rs_b, ALU.mult)
            out_writer(ct, sl, tmp)


def _emit_ln1_writes(tc, pools, pfx, io, mode="both"):
    """LN1 output writer: s8 (fp8, FFN1 input) and/or s16 (bf16 residual).

    mode "s8_pool" writes only s8, on the Pool engine (gelu-era DVE is
    saturated); the s16 pass is then recomputed later via mode "s16".
    """
    nc = tc.nc
    params = io["params"]
    if "s8" not in io:
        io["s8"] = pools["s_pool"].tile([P, CT, HW], FP8, tag=f"s8_{pfx}",
                                        name=f"s8_{pfx}")
        io["s16"] = pools["s_pool"].tile([P, CT, HW], BF16, tag=f"s16_{pfx}",
                                         name=f"s16_{pfx}")
    s8, s16 = io["s8"], io["s16"]

    def _ln1_write(ct, sl, tmp):
        if mode == "s16_to_pool":
            # s8 stays on DVE (critical path to the first gelus); s16
            # rides on Pool so av(7)-normalize starts earlier on DVE.
            nc.vector.tensor_scalar(
                out=s8[:, ct, sl], in0=tmp,
                scalar1=params["n1w"][:, ct:ct + 1],
                scalar2=params["n1b"][:, ct:ct + 1],
                op0=ALU.mult, op1=ALU.add,
            )
            nc.gpsimd.tensor_scalar(
                out=s16[:, ct, sl], in0=tmp,
                scalar1=params["n1w16"][:, ct:ct + 1],
                scalar2=params["n1b16"][:, ct:ct + 1],
                op0=ALU.mult, op1=ALU.add,
            )
            return
        if mode in ("both", "s8"):
            nc.vector.tensor_scalar(
                out=s8[:, ct, sl], in0=tmp,
                scalar1=params["n1w"][:, ct:ct + 1],
                scalar2=params["n1b"][:, ct:ct + 1],
                op0=ALU.mult, op1=ALU.add,
            )
        elif mode == "s8_pool":
            nc.gpsimd.tensor_scalar(
                out=s8[:, ct, sl], in0=tmp,
                scalar1=params["n1w"][:, ct:ct + 1],
                scalar2=params["n1b"][:, ct:ct + 1],
                op0=ALU.mult, op1=ALU.add,
            )
        if mode in ("both", "s16"):
            nc.vector.tensor_scalar(
                out=s16[:, ct, sl], in0=tmp,
                scalar1=params["n1w16"][:, ct:ct + 1],
                scalar2=params["n1b16"][:, ct:ct + 1],
                op0=ALU.mult, op1=ALU.add,
            )

    return _ln1_write


def _emit_ffn1(tc, pools, pfx, io, hts, after_ht=None):
    """FFN1 (fp8 DR, W1 x16) + gelu(scale=1/16) -> h fp8 [P, HT, HW].

    after_ht(ht) lets the caller drip other PE work (e.g. streamed FFN2
    accumulation over the just-finished h tiles) into the emission.
    """
    nc = tc.nc
    params = io["params"]
    w1 = io["w1"]
    if "h" not in io:
        io["h"] = pools["hbuf"].tile([P, HT, HW], FP8, tag="hbuf",
                                     name="hbuf")
    h = io["h"]
    s8 = io["s8"]
    for ht in hts:
        ph = pools["psum_s"].tile([P, HW], F32, tag="s", name="s")
        for ch in range(NCH):
            sl = slice(ch * 512, (ch + 1) * 512)
            for k in range(CT // 2):
                nc.tensor.matmul(
                    ph[:, sl],
                    lhsT=w1[:, 2 * k:2 * k + 2, ht * P:(ht + 1) * P],
                    rhs=s8[:, 2 * k:2 * k + 2, sl],
                    start=(k == 0), stop=(k == CT // 2 - 1),
                    perf_mode=DR,
                )
        nc.scalar.activation(
            out=h[:, ht, :], in_=ph, func=AF.Gelu,
            bias=params["b1"][:, ht:ht + 1], scale=1.0 / 16.0,
        )
        if after_ht is not None:
            after_ht(ht)


def _emit_ffn2_stream_start(tc, pools, io):
    """Held FFN2 ch0 accumulators for ct0/ct1 in the (idle) psum_av banks."""
    io["pfs"] = {
        ct: pools["psum_av"].tile([P, 512], F32, tag="av", name=f"pf{ct}")
        for ct in (0, 1)
    }


def _emit_ffn2_stream_k(tc, pools, io, k):
    """One k-pair of streamed FFN2 ch0 accumulation (needs h 2k,2k+1)."""
    nc = tc.nc
    w2, h = io["w2"], io["h"]
    for ct in (0, 1):
        nc.tensor.matmul(
            io["pfs"][ct],
            lhsT=w2[:, 2 * k:2 * k + 2, ct * P:(ct + 1) * P],
            rhs=h[:, 2 * k:2 * k + 2, 0:512],
            start=(k == 0), stop=(k == HT // 2 - 1),
            perf_mode=DR,
        )


def _emit_ffn2_stream_finish(tc, pools, pfx, io):
    """Evacuate the streamed ct0/ct1 ch0 psums -> r2 rows."""
    nc = tc.nc
    params = io["params"]
    if "r2" not in io:
        io["r2"] = pools["r_pool"].tile([P, CT, HW], BF16, tag=f"r_{pfx}",
                                        name=f"r2_{pfx}")
    for ct in (0, 1):
        nc.vector.scalar_tensor_tensor(
            out=io["r2"][:, ct, 0:512], in0=io["pfs"][ct],
            scalar=params["b216"][:, ct:ct + 1],
            in1=io["s16"][:, ct, 0:512], op0=ALU.add, op1=ALU.add,
        )
    del io["pfs"]


def _emit_ffn2(tc, pools, pfx, io, ct_chs):
    """FFN2 (fp8 DR, W2 x16) + 16*b2 + s16 residual -> r2_bf (=16*r2)."""
    nc = tc.nc
    params = io["params"]
    w2 = io["w2"]
    h = io["h"]
    psum_mm = pools["psum_mm"]
    if "r2" not in io:
        io["r2"] = pools["r_pool"].tile([P, CT, HW], BF16, tag=f"r_{pfx}",
                                        name=f"r2_{pfx}")
    r2_bf = io["r2"]
    for ct, ch in ct_chs:
        sl = slice(ch * 512, (ch + 1) * 512)
        pf = psum_mm.tile([P, 512], F32, tag="mm", name="mm")
        for k in range(HT // 2):
            nc.tensor.matmul(
                pf,
                lhsT=w2[:, 2 * k:2 * k + 2, ct * P:(ct + 1) * P],
                rhs=h[:, 2 * k:2 * k + 2, sl],
                start=(k == 0), stop=(k == HT // 2 - 1),
                perf_mode=DR,
            )
        nc.vector.scalar_tensor_tensor(
            out=r2_bf[:, ct, sl], in0=pf, scalar=params["b216"][:, ct:ct + 1],
            in1=io["s16"][:, ct, sl], op0=ALU.add, op1=ALU.add,
        )


def _emit_ln2_norm(tc, pools, pfx, io, ch, act_scale=False):
    """LN2 normalize+scale+DMA out for one chunk.

    act_scale=True puts the *w+b step on ACT (Identity, table-free) --
    only for tail chunks where ACT is otherwise idle.
    """
    nc = tc.nc
    params, out_dram = io["params"], io["out"]
    st = io["ln2st"]

    def _ln2_write(ct, sl, tmp):
        o32 = pools["ostage"].tile([P, 512], F32, tag="ostage", name="ostage",
                                   bufs=2)
        if act_scale:
            nc.scalar.activation(
                out=o32, in_=tmp, func=AF.Identity,
                bias=params["n2b"][:, ct:ct + 1],
                scale=params["n2w"][:, ct:ct + 1],
            )
        else:
            nc.vector.tensor_scalar(
                out=o32, in0=tmp,
                scalar1=params["n2w"][:, ct:ct + 1],
                scalar2=params["n2b"][:, ct:ct + 1],
                op0=ALU.mult, op1=ALU.add,
            )
        eng = (nc.sync, nc.gpsimd, nc.scalar, nc.sync)[ct % 4]
        eng.dma_start(out=out_dram[ct * P:(ct + 1) * P, sl], in_=o32)

    _emit_ln_norm(tc, pools, io["r2"], st, _ln2_write, chunks=(ch,))


def _emit_ln2_stats(tc, pools, pfx, io, ch, sq_dve=True, rsqrt=True):
    st = io.setdefault("ln2st", {})
    _emit_ln_stats(tc, pools, io["r2"], st, io["inv512"], io["eps"],
                   chunks=(ch,), sq_dve=sq_dve, rsqrt=rsqrt)


def build_program():
    nc = bacc.Bacc("TRN2", target_bir_lowering=False, debug=False)

    def din(name, shape, dt):
        return nc.dram_tensor(name, list(shape), dt, kind="ExternalInput").ap()

    x8 = {p: din(f"x_{p}8", (P, CT, HW), FP8) for p in "sf"}
    x256 = {p: din(f"x_{p}256", (P, CT, HW), BF16) for p in "sf"}
    wq8 = {p: din(f"{p}_wq8", (P, CT, C), FP8) for p in "sf"}
    wk8 = {p: din(f"{p}_wk8", (P, CT, C), FP8) for p in "sf"}
    wv8 = {p: din(f"{p}_wv8", (P, CT, C), FP8) for p in "sf"}
    wo8 = {p: din(f"{p}_wo8", (P, CT, C), FP8) for p in "sf"}
    w18 = {p: din(f"{p}_w18", (P, CT, HID), FP8) for p in "sf"}
    w28 = {p: din(f"{p}_w28", (P, HT, C), FP8) for p in "sf"}
    prm_d = din("prm", (P, 2, NPCOL), F32)
    outs = {
        p: nc.dram_tensor(f"out_{p}", [C, HW], F32, kind="ExternalOutput").ap()
        for p in "sf"
    }

    with tile.TileContext(nc) as tc:
        from contextlib import ExitStack
        with ExitStack() as ctx:
            pools = {}

            def pool(name, bufs, space="SBUF", stack=None):
                pools[name] = (stack or ctx).enter_context(
                    tc.tile_pool(name=name, bufs=bufs, space=space)
                )
                return pools[name]

            # whole-program pools
            pool("psum_mm", 2, space="PSUM")
            pool("psum_s", 2, space="PSUM")
            pool("psum_av", 2, space="PSUM")
            pool("consts", 1)
            pool("params", 1)
            pool("rows", 1)
            pool("xr", 2)
            pool("bcast", 1)
            pool("tmp", 1)
            pool("sq", 2)
            pool("rz", 1)
            pool("pt", 3)
            pool("qkv", 1)
            pool("o_pool", 1)
            pool("wo_pool", 1)
            pool("r_pool", 1)
            pool("s_pool", 1)
            pool("ostage", 2)
            # x8 + wproj live only through the projections; their SBUF is
            # reused by x256/wffn/hbuf afterwards (LIFO close below).
            xw_stack = ctx.enter_context(ExitStack())
            pool("x8", 1, stack=xw_stack)
            pool("wproj", 2, stack=xw_stack)

            inv512 = pools["consts"].tile([P, 1], BF16)
            nc.vector.memset(inv512, 1.0 / C)
            eps_sb = pools["consts"].tile([1, 1], F32)
            nc.vector.memset(eps_sb, EPS)
            # Pin the ACT table set to natural_log_exp_and_others (Ln+Exp)
            # before the softmax exps start.
            lnpin = pools["consts"].tile([1, 1], F32)
            nc.vector.memset(lnpin, 1.0)
            nc.scalar.activation(lnpin, lnpin, AF.Ln, bias=eps_sb[:, 0:1])

            # PE warm-up: junk matmuls spend the p-state ramp (~3us to full
            # clock) while the first DMAs land, so the real projections run
            # at full speed. Nothing reads the psum tiles.
            warm = pools["consts"].tile([P, 2, 512], FP8)
            nc.vector.memset(warm, 0.0625)
            for _ in range(22):
                pw = pools["psum_mm"].tile([P, 512], F32, tag="mm",
                                           name="warm")
                nc.tensor.matmul(pw, lhsT=warm[:, :, 0:P], rhs=warm,
                                 start=True, stop=True, perf_mode=DR)

            # ---- bulk loads: one contiguous DMA each, rotating queues ----
            _q = [nc.sync, nc.gpsimd, nc.scalar]
            _qi = [0]

            def dma_next(out, in_):
                eng = _q[_qi[0] % len(_q)]
                _qi[0] += 1
                eng.dma_start(out=out, in_=in_)

            x8_sb = {}
            w_sb = {}
            for p in "sf":
                x8_sb[p] = pools["x8"].tile([P, CT, HW], FP8, tag=f"x8_{p}",
                                            name=f"x8_{p}")
            for nm, src in (("wq_s", wq8["s"]), ("wk_s", wk8["s"])):
                w_sb[nm] = pools["wproj"].tile([P, CT, C], FP8, tag=nm[:2],
                                               name=nm)
            # head-critical loads first: x_s, wq_s, x_f, wk_s
            dma_next(x8_sb["s"], x8["s"])
            dma_next(w_sb["wq_s"], wq8["s"])
            dma_next(x8_sb["f"], x8["f"])
            dma_next(w_sb["wk_s"], wk8["s"])

            # params block (single small DMA)
            prm_sb = pools["params"].tile([P, 2, NPCOL], F32, tag="prm")
            dma_next(prm_sb, prm_d)
            params = {}
            for pi, p in enumerate("sf"):
                params[p] = {}
                for i, n in enumerate(PNAMES):
                    params[p][n] = prm_sb[:, pi, i * CT:(i + 1) * CT]
                params[p]["b1"] = prm_sb[:, pi, len(PNAMES) * CT:NPCOL]

            qkv = {}
            for p in "sf":
                for g in range(2):
                    qkv[f"q_{p}{g}"] = pools["qkv"].tile(
                        [P, 2, HW], FP8, tag=f"q_{p}{g}", name=f"q_{p}{g}")
                    qkv[f"k_{p}{g}"] = pools["qkv"].tile(
                        [P, 2, HW], FP8, tag=f"k_{p}{g}", name=f"k_{p}{g}")
                qkv[f"v_{p}"] = pools["qkv"].tile(
                    [P, TT, HEADS, VW], FP8, tag=f"v_{p}", name=f"v_{p}")

            wo_sb = {}
            o_sb = {}
            for p in "sf":
                wo_sb[p] = pools["wo_pool"].tile([P, CT, C], FP8,
                                                 tag=f"wo_{p}", name=f"wo_{p}")
                o_sb[p] = pools["o_pool"].tile([P, HEADS // 2, HW], FP8,
                                               tag=f"o_{p}", name=f"o_{p}")

            ios = {}
            for p in "sf":
                ios[p] = {
                    "o": o_sb[p], "wo": wo_sb[p],
                    "params": params[p], "out": outs[p],
                    "inv512": inv512, "eps": eps_sb,
                }

            # software-pipelined attention: S^T+exp of pair N overlaps
            # AV of pair N-1 on PE, so PE never waits on the ACT exp chain.
            # stream 's': q from x_s, kv from x_f ; stream 'f': swapped
            seq = [("s", hp) for hp in range(4)] + [("f", hp) for hp in range(4)]
            pts = {}

            def st(i, filler=None):
                p, hp = seq[i]
                g = hp // 2
                pts[i] = _emit_st_exp(tc, pools, hp, qkv[f"q_{p}{g}"],
                                      qkv[f"k_{p}{g}"], filler=filler)

            def av(i):
                p, hp = seq[i]
                _emit_av(tc, pools, hp, pts.pop(i), qkv[f"v_{p}"], o_sb[p])

            qs2 = [qkv["q_s0"], qkv["q_s1"]]
            ks2 = [qkv["k_s0"], qkv["k_s1"]]
            qf2 = [qkv["q_f0"], qkv["q_f1"]]
            kf2 = [qkv["k_f0"], qkv["k_f1"]]

            # ---- A(s): group g0 first so st(0)/st(1) start early ----
            _emit_proj_qk_one(tc, pools, x8_sb["s"], w_sb["wq_s"], qs2,
                              ts=(0, 1), act_evict=True)
            _emit_proj_qk_one(tc, pools, x8_sb["f"], w_sb["wk_s"], ks2,
                              ts=(0, 1), act_evict=True)
            st(0)
            _emit_proj_qk_one(tc, pools, x8_sb["s"], w_sb["wq_s"], qs2,
                              ts=(2, 3))
            _emit_proj_qk_one(tc, pools, x8_sb["f"], w_sb["wk_s"], ks2,
                              ts=(2, 3))
            st(1)
            # next round of loads
            w_sb["wv_s"] = pools["wproj"].tile([P, CT, C], FP8, tag="wv",
                                               name="wv_s")
            w_sb["wq_f"] = pools["wproj"].tile([P, CT, C], FP8, tag="wq",
                                               name="wq_f")
            w_sb["wk_f"] = pools["wproj"].tile([P, CT, C], FP8, tag="wk",
                                               name="wk_f")
            w_sb["wv_f"] = pools["wproj"].tile([P, CT, C], FP8, tag="wv",
                                               name="wv_f")
            dma_next(w_sb["wv_s"], wv8["s"])
            dma_next(w_sb["wq_f"], wq8["f"])
            dma_next(w_sb["wk_f"], wk8["f"])
            nc.vector.memset(qkv["v_s"][:, :, :, DH + 1:], 0.0)
            _emit_proj_v(tc, pools, x8_sb["f"], w_sb["wv_s"], qkv["v_s"])
            av(0)
            st(2)
            dma_next(w_sb["wv_f"], wv8["f"])
            dma_next(wo_sb["s"], wo8["s"])
            dma_next(wo_sb["f"], wo8["f"])
            av(1)
            _emit_proj_qk_one(tc, pools, x8_sb["f"], w_sb["wq_f"], qf2,
                              ts=(0, 1))
            _emit_proj_qk_one(tc, pools, x8_sb["s"], w_sb["wk_f"], kf2,
                              ts=(0, 1))
            st(3)
            av(2)
            _emit_proj_qk_one(tc, pools, x8_sb["f"], w_sb["wq_f"], qf2,
                              ts=(2, 3))
            _emit_proj_qk_one(tc, pools, x8_sb["s"], w_sb["wk_f"], kf2,
                              ts=(2, 3))
            nc.vector.memset(qkv["v_f"][:, :, :, DH + 1:], 0.0)
            _emit_proj_v(tc, pools, x8_sb["s"], w_sb["wv_f"], qkv["v_f"])
            # projections done: free x8/wproj SBUF for wffn/hbuf
            xw_stack.close()
            pool("wffn", 1)
            pool("hbuf", 1)
            for p in "sf":
                ios[p]["x256d"] = x256[p]
            ios["s"]["w1"] = pools["wffn"].tile([P, CT, HID], FP8, tag="w1",
                                                name="w1_s", bufs=1)
            ios["s"]["w2"] = pools["wffn"].tile([P, HT, C], FP8, tag="w2",
                                                name="w2_s", bufs=2)
            dma_next(ios["s"]["w1"], w18["s"])
            dma_next(ios["s"]["w2"], w28["s"])
            # av/wo/LN1(s) ride as fillers INSIDE the st blocks: their PE
            # work lands in the PE slack of the block instead of delaying
            # the next block's S^T matmuls (which would starve the exps).
            st(4)
            av(3)
            st(5)
            _emit_wo_residual(tc, pools, "s", ios["s"], (0, 1))
            av(4)
            st(6)
            _emit_wo_residual(tc, pools, "s", ios["s"], (2, 3))
            av(5)
            # LN1(s) fully inside the attention era: rsqrt shares the
            # softmax Ln+Exp table set, the normalize rides the idle DVE
            # slack, so s8 is ready the moment the last exp retires.
            ios["s"]["ln1st"] = {}
            _emit_ln_stats(tc, pools, ios["s"]["r"], ios["s"]["ln1st"],
                           inv512, eps_sb, rsqrt=True)
            st(7)
            av(6)
            _emit_ln_norm(tc, pools, ios["s"]["r"], ios["s"]["ln1st"],
                          _emit_ln1_writes(tc, pools, "s", ios["s"]))
            av(7)

            # ---- gelu era. ACT order: gelus(s), ln1(f), gelus(f),
            # ln2(s,0), ln2(s,1), ln2(f,0), ln2(f,1) -> 4 table reloads.
            # FFN2 ch0 ct0/ct1 accumulate in the idle psum_av banks DURING
            # the gelus (k-pair k right after gelu 2k+1), so only ct2/ct3
            # remain after the last gelu. gelus(s) start right after the
            # last exp; wo_residual(f) runs on PE behind the FFN1(s) mms.
            # DVE is the scarce engine during gelus(s): LN1(f) writes only
            # s8 there (TSPs on Pool); the s16 pass is recomputed during
            # gelus(f) when DVE is light again.
            _emit_ffn2_stream_start(tc, pools, ios["s"])
            ios["f"]["ln1st"] = {}

            def s_after_ht(ht):
                # Only drip PE work whose deps are certainly met at this
                # queue position (PE is in-order: a stalled drip matmul
                # blocks the FFN1 matmuls and starves the gelus).
                if ht % 2 == 1:
                    _emit_ffn2_stream_k(tc, pools, ios["s"], ht // 2)

            _emit_ffn1(tc, pools, "s", ios["s"], range(0, 16),
                       after_ht=s_after_ht)
            _emit_wo_residual(tc, pools, "f", ios["f"], (0, 1))
            _emit_wo_residual(tc, pools, "f", ios["f"], (2, 3))
            _emit_ln_stats(tc, pools, ios["f"]["r"], ios["f"]["ln1st"],
                           inv512, eps_sb, rsqrt=True)
            _emit_ln_norm(tc, pools, ios["f"]["r"], ios["f"]["ln1st"],
                          _emit_ln1_writes(tc, pools, "f", ios["f"],
                                           mode="s8_pool"))
            _emit_ffn2_stream_finish(tc, pools, "s", ios["s"])
            _emit_ffn2(tc, pools, "s", ios["s"], [(2, 0), (3, 0)])
            _emit_ln2_stats(tc, pools, "s", ios["s"], 0, rsqrt=False)
            ios["f"]["w1"] = pools["wffn"].tile([P, CT, HID], FP8, tag="w1",
                                                name="w1_f", bufs=1)
            dma_next(ios["f"]["w1"], w18["f"])
            ios["f"]["w2"] = pools["wffn"].tile([P, HT, C], FP8, tag="w2",
                                                name="w2_f", bufs=2)
            dma_next(ios["f"]["w2"], w28["f"])
            # f-stream gelus; FFN2(f) ch0 streams into psum_av, and the
            # four FFN2(s) ch1 column blocks drip in behind h_s (complete).
            _emit_ffn2_stream_start(tc, pools, ios["f"])

            def f_after_ht(ht):
                if ht % 2 == 1:
                    _emit_ffn2_stream_k(tc, pools, ios["f"], ht // 2)
                    if ht <= 7:
                        _emit_ffn2(tc, pools, "s", ios["s"],
                                   [((ht - 1) // 2, 1)])

            _emit_ffn1(tc, pools, "f", ios["f"], range(0, 16),
                       after_ht=f_after_ht)
            _emit_ln_rsqrt(tc, pools, ios["s"]["ln2st"], eps_sb, chunks=(0,))
            _emit_ln2_norm(tc, pools, "s", ios["s"], 0)
            _emit_ln2_stats(tc, pools, "s", ios["s"], 1)
            # s16(f) recompute sits AFTER ln2(s,1) stats on DVE (it is only
            # needed by the FFN2(f) stream-finish stts) but BEFORE the
            # ln2(f) stats allocate their row tiles.
            _emit_ln_norm(tc, pools, ios["f"]["r"], ios["f"]["ln1st"],
                          _emit_ln1_writes(tc, pools, "f", ios["f"],
                                           mode="s16"))
            _emit_ln2_norm(tc, pools, "s", ios["s"], 1)
            _emit_ffn2_stream_finish(tc, pools, "f", ios["f"])
            _emit_ffn2(tc, pools, "f", ios["f"], [(2, 0), (3, 0)])
            _emit_ln2_stats(tc, pools, "f", ios["f"], 0)
            _emit_ln2_norm(tc, pools, "f", ios["f"], 0, act_scale=True)
            _emit_ffn2(tc, pools, "f", ios["f"],
                       [(ct, 1) for ct in range(CT)])
            _emit_ln2_stats(tc, pools, "f", ios["f"], 1)
            _emit_ln2_norm(tc, pools, "f", ios["f"], 1, act_scale=True)

    nc.compile()
    return nc


# --------------------------------------------------------------------------
# host side
# --------------------------------------------------------------------------

_BF = ml_dtypes.bfloat16
_F8 = ml_dtypes.float8_e4m3
WS = 16.0  # host weight scale for fp8 matmuls
XS = WS * WS  # attention product scale absorbed by LN1


def _head_perm():
    """Permuted output-channel order for Q/K projections.

    Tile t = 2g+hi, partition 32*h4+lo  ->  orig channel (4g+h4)*64+32*hi+lo.
    """
    perm = np.zeros(C, dtype=np.int64)
    i = 0
    for g in range(2):
        for hi in range(2):
            for h4 in range(4):
                for lo in range(32):
                    perm[i] = (4 * g + h4) * 64 + 32 * hi + lo
                    i += 1
    return perm


def _pmajor(m, tiles):
    """[tiles*P, X] -> [P, tiles, X] with row c = t*P + p."""
    return np.ascontiguousarray(
        m.reshape(tiles, P, m.shape[1]).transpose(1, 0, 2)
    )


def _prep_shared_inputs(inputs):
    """Host-side weight prep: transposes, casts, permutations, x16 scales."""
    sh = {}
    perm = _head_perm()
    prm = np.zeros((P, 2, NPCOL), np.float32)
    for pi, (p, ap) in enumerate((("s", "s_"), ("f", "f_"))):
        wq, wk, wv, wo = (inputs[ap + n] for n in ("Wq", "Wk", "Wv", "Wo"))
        sh[f"{p}_wq8"] = _pmajor((wq.T[:, perm] * WS).astype(_F8), CT)
        sh[f"{p}_wk8"] = _pmajor((wk.T[:, perm] * WS).astype(_F8), CT)
        sh[f"{p}_wv8"] = _pmajor((wv.T * WS).astype(_F8), CT)
        sh[f"{p}_wo8"] = _pmajor((wo.T * WS).astype(_F8), CT)
        w1 = inputs[f"{p}ffn_W1"]
        w2 = inputs[f"{p}ffn_W2"]
        sh[f"{p}_w18"] = _pmajor((w1.T * WS).astype(_F8), CT)
        sh[f"{p}_w28"] = _pmajor((w2.T * WS).astype(_F8), HT)
        vals = {
            "bo256": inputs[ap + "bo"] * XS,
            "n1w": inputs[f"{p}n1_w"], "n1b": inputs[f"{p}n1_b"],
            "n1w16": inputs[f"{p}n1_w"] * WS,
            "n1b16": inputs[f"{p}n1_b"] * WS,
            "n2w": inputs[f"{p}n2_w"], "n2b": inputs[f"{p}n2_b"],
            "b216": inputs[f"{p}ffn_b2"] * WS,
        }
        for i, n in enumerate(PNAMES):
            prm[:, pi, i * CT:(i + 1) * CT] = vals[n].reshape(CT, P).T
        prm[:, pi, len(PNAMES) * CT:NPCOL] = (
            inputs[f"{p}ffn_b1"].reshape(HT, P).T
        )
    sh["prm"] = prm
    return sh


def make_in_maps(inputs):
    shared = _prep_shared_inputs(inputs)
    xs = np.ascontiguousarray(inputs["spatial_feat"].reshape(B, C, HW))
    xf = np.ascontiguousarray(inputs["freq_feat"].reshape(B, C, HW))
    in_maps = []
    for b in range(N_CORES):
        m = dict(shared)
        m["x_s8"] = _pmajor(xs[b].astype(_F8), CT)
        m["x_f8"] = _pmajor(xf[b].astype(_F8), CT)
        m["x_s256"] = _pmajor((xs[b] * XS).astype(_BF), CT)
        m["x_f256"] = _pmajor((xf[b] * XS).astype(_BF), CT)
        in_maps.append(m)
    return in_maps


_CACHED = {}


def _get_program():
    if "nc" not in _CACHED:
        _CACHED["nc"] = build_program()
    return _CACHED["nc"]


def run_on_hw(inputs, trace=False, trace_kwargs=None):
    from concourse.bass_utils import run_bass_kernel_spmd

    nc = _get_program()
    in_maps = make_in_maps(inputs)
    res = run_bass_kernel_spmd(
        nc, in_maps, list(range(N_CORES)), trace=trace,
        **(dict(trace_kwargs=trace_kwargs) if trace_kwargs else {}),
    )
    s = np.stack([res.results[b]["out_s"] for b in range(B)])
    f = np.stack([res.results[b]["out_f"] for b in range(B)])
    s = s.reshape(B, C, H_IMG, W_IMG).astype(np.float32)
    f = f.reshape(B, C, H_IMG, W_IMG).astype(np.float32)
    return (s, f), res


def kernel(**inputs):
    out, _ = run_on_hw(inputs, trace=False)
    return out


if __name__ == "__main__":
    import reference

    inputs = {k: np.asarray(v) for k, v in reference.setup_inputs().items()}
    exp_s, exp_f = reference.reference(**inputs)
    act_s, act_f = kernel(**inputs)
    for nm, e, a in (("s", exp_s, act_s), ("f", exp_f, act_f)):
        err = np.abs(np.asarray(a) - np.asarray(e)).max()
        print(nm, "absmax", err, "rel", err / np.abs(e).max())


# revision 40
# speedup vs baseline: 1.0010x; 1.0010x over previous
"""CrossFusionBlock Trainium2 kernel.

Dual-stream cross-attention block (B=8, C=512, HW=1024, 8 heads, FFN 2048).
Sharding: data-parallel over batch across 8 NeuronCores (1 batch element per
core), weights replicated. All weight transposes / casts / permutations are
done on the host so the device kernel contains no transposes at all.

v2: ACT (scalar engine) is the bottleneck (softmax exp 131k lanes-cycles +
gelu). Everything else is scheduled around keeping ACT saturated:
  - all projections fp8 DoubleRow (matmul cost = out-cols only, fp8 DR
    halves it); proj weights host-scaled x16 so fp8 e4m3 is well covered.
    Attention product scale 16*16=256 absorbed by LN1 (x256 residual).
  - p-major host layouts: every weight/activation load is one contiguous
    DMA; params ride in a single [P,2,48] block. First exp at ~5us.
  - softmax normalize: reciprocal writes partition 0 (misaligned DVE op),
    Pool partition_broadcast fans out 1/Z, both head-halves written by DVE
    tensor ops (par1 with misaligned out partitions) -- no DMAs at all.
  - LN mu/rs broadcasts on Pool partition_broadcast instead of DMA.
  - ACT order: [exps | LN1(s) rsqrt inline (shared Ln+Exp table set),
    gelus(s), LN1(f) rsqrt, gelus(f), LN2(s+f) rsqrt] -> 4 table reloads,
    no ACT stalls. LN1(f) stats emitted early, its ACT part late.
  - FFN2/LN2 chunk-interleaved so the tail chain is one half-stream deep.
"""

import sys

import numpy as np

for _p in ("/opt/trn_rl_repo", "/opt/pypackages"):
    if _p not in sys.path:
        sys.path.insert(0, _p)

import ml_dtypes  # noqa: E402

import concourse.bass as bass  # noqa: E402
from concourse import bacc  # noqa: E402
import concourse.mybir as mybir  # noqa: E402
import concourse.tile as tile  # noqa: E402


def _patch_act_tables():
    """Make natural_log_exp_and_others the only set offering Exp/Ln.

    The table-load pass greedily picks the first set containing each
    activation function, which ping-pongs between the ln-only and exp-only
    sets (2 table loads per LayerNorm rsqrt). Hiding Exp/Ln from the other
    sets forces the combined set; set ids keep their true act_info indices
    so the emitted LoadActFuncSet ids stay valid for walrus.
    """
    import concourse.hw_specs as hw_specs

    if getattr(hw_specs, "_act_tables_patched", False):
        return
    orig = hw_specs.get_activation_tables

    def patched(arch):
        tabs = dict(orig(arch))
        exp = mybir.ActivationFunctionType.Exp
        ln = mybir.ActivationFunctionType.Ln
        out = {}
        for name, funcs in tabs.items():
            if name != "natural_log_exp_and_others":
                funcs = funcs - {exp, ln}
            out[name] = funcs
        return out

    hw_specs._act_tables_patched = True
    hw_specs.get_activation_tables = patched
    bacc.get_activation_tables = patched


_patch_act_tables()

P = 128
C = 512
HW = 1024
HEADS = 8
DH = 64
HID = 2048
CT = C // P        # 4 channel tiles
HT = HID // P      # 16 hidden tiles
TT = HW // P       # 8 token tiles
NCH = HW // 512    # 2 free-dim chunks of 512
EPS = 1e-6
BF16 = mybir.dt.bfloat16
FP8 = mybir.dt.float8e4
F32 = mybir.dt.float32
AF = mybir.ActivationFunctionType
ALU = mybir.AluOpType
DR = mybir.MatmulPerfMode.DoubleRow

N_CORES = 8
B, H_IMG, W_IMG = 8, 32, 32

VW = 72  # V row width: DH + ones col + zero pad (16B-aligned for DoubleRow)

# param block column layout: 8 x [P, CT] then b1 [P, HT]
PNAMES = ("bo256", "n1w", "n1b", "n1w16", "n1b16", "n2w", "n2b", "b216")
NPCOL = len(PNAMES) * CT + HT  # 48


# --------------------------------------------------------------------------
# device program
# --------------------------------------------------------------------------

def _emit_proj_qk_one(tc, pools, x8, w8, out2, ts=tuple(range(CT)),
                      act_evict=False):
    """fp8 DR projection with head-grouped permuted W -> out2 = [g0, g1].

    Psum tile t = 2*g + hi holds rows (head 4g+h4, d = 32*hi + lo) at
    partition 32*h4 + lo; evacuated to out2[g][:, hi, :]. act_evict
    alternates psum eviction between ACT and DVE -- only useful in the
    head, while ACT still has no exps to chew on.
    """
    nc = tc.nc
    psum_mm = pools["psum_mm"]
    i = 0
    for t in ts:
        g, hi = t // 2, t % 2
        for ch in range(NCH):
            pq = psum_mm.tile([P, 512], F32, tag="mm", name="mm")
            for k in range(CT // 2):
                nc.tensor.matmul(
                    pq,
                    lhsT=w8[:, 2 * k:2 * k + 2, t * P:(t + 1) * P],
                    rhs=x8[:, 2 * k:2 * k + 2, ch * 512:(ch + 1) * 512],
                    start=(k == 0), stop=(k == CT // 2 - 1),
                    perf_mode=DR,
                )
            dst = out2[g][:, hi, ch * 512:(ch + 1) * 512]
            if act_evict and i % 2 == 0:
                nc.scalar.copy(out=dst, in_=pq)
            else:
                nc.vector.tensor_copy(out=dst, in_=pq)
            i += 1


def _emit_proj_v(tc, pools, xkv8, wv8, v_hf):
    nc = tc.nc
    psum_mm = pools["psum_mm"]
    for tt in range(TT):
        pv = psum_mm.tile([P, 512], F32, tag="mm", name="mm")
        for k in range(CT // 2):
            nc.tensor.matmul(
                pv,
                lhsT=xkv8[:, 2 * k:2 * k + 2, tt * P:(tt + 1) * P],
                rhs=wv8[:, 2 * k:2 * k + 2, :],
                start=(k == 0), stop=(k == CT // 2 - 1),
                perf_mode=DR,
            )
        nc.vector.tensor_copy(
            out=v_hf[:, tt, :, 0:DH],
            in_=pv.rearrange("p (h d) -> p h d", d=DH),
        )
        nc.vector.memset(v_hf[:, tt, :, DH:DH + 1], 1.0)


def _emit_st_exp(tc, pools, hp, q2g, k2g, filler=None):
    """S^T per head via fp8 DoubleRow (Ki=32 x2) -> exp(P^T*2^-11) fp8.

    filler() is emitted after tt==1: its PE work rides in the huge PE
    slack inside the st block instead of BETWEEN st blocks, where it
    would delay the next block's S^T matmuls and starve the exp stream.
    """
    nc = tc.nc
    pt = {}
    for par in (0, 1):
        pt[par] = pools["pt"].tile([P, TT, HW], FP8, tag="pt", name="pt",
                                   bufs=4)
    for tt in range(TT):
        if tt == 2 and filler is not None:
            filler()
        ps = {}
        for par in (0, 1):
            h4 = (2 * hp + par) % 4
            base = 32 * h4
            kw = {"tile_position": (96, 0)} if h4 == 3 else {}
            p_s = pools["psum_s"].tile([P, HW], F32, tag="s", name="s")
            for ch in range(NCH):
                nc.tensor.matmul(
                    p_s[:, ch * 512:(ch + 1) * 512],
                    lhsT=k2g[base:base + 32, :, tt * P:(tt + 1) * P],
                    rhs=q2g[base:base + 32, :, ch * 512:(ch + 1) * 512],
                    start=True, stop=True,
                    perf_mode=DR,
                    **kw,
                )
            ps[par] = p_s
        for par in (0, 1):
            nc.scalar.activation(out=pt[par][:, tt, :], in_=ps[par],
                                 func=AF.Exp, scale=0.125 / 256.0)
    return pt


def _emit_av(tc, pools, hp, pt, v_hf, o_pair):
    """AV+Z (ones column) fp8 DR -> normalize into o_pair[:, hp].

    Z sits at psum row DH; its reciprocal is written to partition 0 of a
    [1,512] tile (misaligned DVE op), Pool broadcasts it to 64 partitions,
    then both head-halves are normalized by DVE tensor ops (par1 writes
    partitions 64:128 from psum rows 0:64 -- misaligned out).
    """
    nc = tc.nc
    for par in (0, 1):
        h = 2 * hp + par
        for ch in range(NCH):
            sl = slice(ch * 512, (ch + 1) * 512)
            pav = pools["psum_av"].tile([P, 512], F32, tag="av", name="av")
            for tt2 in range(TT // 2):
                nc.tensor.matmul(
                    pav[0:VW, :],
                    lhsT=v_hf[:, 2 * tt2:2 * tt2 + 2, h, :],
                    rhs=pt[par][:, 2 * tt2:2 * tt2 + 2, sl],
                    start=(tt2 == 0), stop=(tt2 == TT // 2 - 1),
                    perf_mode=DR,
                )
            rz0 = pools["rz"].tile([1, 512], BF16, tag="rz0", name="rz0",
                                   bufs=4)
            with nc.allow_low_precision(reason="1/Z feeds an fp8 store"):
                nc.vector.reciprocal(out=rz0, in_=pav[DH:DH + 1, :])
            rzb = pools["rz"].tile([DH, 512], BF16, tag="rzb", name="rzb",
                                   bufs=4)
            nc.gpsimd.partition_broadcast(rzb, rz0, channels=DH)
            nc.vector.tensor_tensor(
                o_pair[par * DH:(par + 1) * DH, hp, sl],
                pav[0:DH, :], rzb, ALU.mult,
            )


def _emit_wo_residual(tc, pools, pfx, io, cts):
    """Wo(16x fp8) projection + 256*bo + 256*x residual -> r_bf (=256*r)."""
    nc = tc.nc
    o_hf = io["o"]
    wo, params = io["wo"], io["params"]
    psum_mm = pools["psum_mm"]
    if "r" not in io:
        io["r"] = pools["r_pool"].tile([P, CT, HW], BF16, tag=f"r_{pfx}",
                                       name=f"r_{pfx}")
    r_bf = io["r"]
    for ct in cts:
        x256 = pools["xr"].tile([P, HW], BF16, tag="xr", name="xr", bufs=2)
        nc.gpsimd.dma_start(out=x256, in_=io["x256d"][:, ct, :])
        for ch in range(NCH):
            sl = slice(ch * 512, (ch + 1) * 512)
            pe_ = psum_mm.tile([P, 512], F32, tag="mm", name="mm")
            for i2 in range(HEADS // 4):
                nc.tensor.matmul(
                    pe_,
                    lhsT=wo[:, 2 * i2:2 * i2 + 2, ct * P:(ct + 1) * P],
                    rhs=o_hf[:, 2 * i2:2 * i2 + 2, sl],
                    start=(i2 == 0), stop=(i2 == HEADS // 4 - 1),
                    perf_mode=DR,
                )
            nc.vector.scalar_tensor_tensor(
                out=r_bf[:, ct, sl], in0=pe_,
                scalar=params["bo256"][:, ct:ct + 1],
                in1=x256[:, sl], op0=ALU.add, op1=ALU.add,
            )


def _emit_ln_stats(tc, pools, src_bf, st, inv512, eps_sb,
                   chunks=tuple(range(NCH)), sq_dve=False, rsqrt=True):
    """LN stats over the channel (partition x 4-tile) axis of [P, CT, HW].

    Fills st["mu2"]/st["rs2"] rows ([1, NCH, 512]). rsqrt=True also emits
    the ACT Ln+Exp pair per chunk; rsqrt=False defers it to
    _emit_ln_rsqrt (so the ACT ops can be queued later).
    """
    nc = tc.nc
    psum_mm = pools["psum_mm"]
    if "mu2" not in st:
        st["mu2"] = pools["rows"].tile([1, NCH, 512], BF16, tag="mu2",
                                       name="mu2", bufs=2)
        st["var2"] = pools["rows"].tile([1, NCH, 512], F32, tag="var2",
                                        name="var2", bufs=2)
        st["rs2"] = pools["rows"].tile([1, NCH, 512], BF16, tag="rs2",
                                       name="rs2", bufs=2)
    mu2, var2, rs2 = st["mu2"], st["var2"], st["rs2"]
    for ch in chunks:
        sl = slice(ch * 512, (ch + 1) * 512)
        pmu = psum_mm.tile([1, 512], F32, tag="mm", name="mm")
        for k in range(CT):
            nc.tensor.matmul(
                pmu, lhsT=inv512[:, 0:1], rhs=src_bf[:, k, sl],
                start=(k == 0), stop=(k == CT - 1),
            )
        pms = psum_mm.tile([1, 512], F32, tag="mm", name="mm")
        for k in range(CT):
            r2 = pools["sq"].tile([P, 512], BF16, tag="sq", name="sq")
            if sq_dve == "alt":
                sq_eng = nc.vector if k % 2 else nc.gpsimd
            else:
                sq_eng = nc.vector if sq_dve else nc.gpsimd
            sq_eng.tensor_tensor(r2, src_bf[:, k, sl], src_bf[:, k, sl],
                                 ALU.mult)
            nc.tensor.matmul(
                pms, lhsT=inv512[:, 0:1], rhs=r2,
                start=(k == 0), stop=(k == CT - 1),
            )
        nc.vector.tensor_copy(out=mu2[0:1, ch, :], in_=pmu)
        musq = pools["rows"].tile([1, 512], F32, tag="musq", name="musq",
                                  bufs=2)
        nc.vector.tensor_tensor(musq, mu2[0:1, ch, :], mu2[0:1, ch, :],
                                ALU.mult)
        nc.vector.tensor_tensor(var2[0:1, ch, :], pms, musq, ALU.subtract)
        if rsqrt:
            _emit_ln_rsqrt(tc, pools, st, eps_sb, chunks=(ch,))


def _emit_ln_rsqrt(tc, pools, st, eps_sb, chunks=tuple(range(NCH))):
    """rs = 1/sqrt(var+eps) = exp(-0.5*ln(var+eps)) on ACT (Ln+Exp set)."""
    nc = tc.nc
    for ch in chunks:
        lnv = pools["rows"].tile([1, 512], F32, tag="lnv", name="lnv", bufs=2)
        nc.scalar.activation(lnv, st["var2"][0:1, ch, :], AF.Ln,
                             bias=eps_sb[:, 0:1])
        nc.scalar.activation(st["rs2"][0:1, ch, :], lnv, AF.Exp, scale=-0.5)


def _emit_ln_norm(tc, pools, src_bf, st, out_writer,
                  chunks=tuple(range(NCH)), pool_cts=()):
    """Broadcast mu/rs (Pool) and hand normalized [P,512] bf16 pieces on.

    pool_cts: ct indices whose subtract/mult pair runs on Pool instead of
    DVE (load-balancing for windows where DVE is saturated).
    """
    nc = tc.nc
    for ch in chunks:
        sl = slice(ch * 512, (ch + 1) * 512)
        mu_b = pools["bcast"].tile([P, 512], BF16, tag="mu_b", name="mu_b",
                                   bufs=2)
        rs_b = pools["bcast"].tile([P, 512], BF16, tag="rs_b", name="rs_b",
                                   bufs=2)
        nc.gpsimd.partition_broadcast(mu_b, st["mu2"][0:1, ch, :], channels=P)
        nc.gpsimd.partition_broadcast(rs_b, st["rs2"][0:1, ch, :], channels=P)
        for ct in range(CT):
            eng = nc.gpsimd if ct in pool_cts else nc.vector
            tmp = pools["tmp"].tile([P, 512], BF16, tag="tmp", name="tmp",
                                    bufs=2)
            eng.tensor_tensor(tmp, src_bf[:, ct, sl], mu_b, ALU.subtract)
            eng.tensor_tensor(tmp, tmp, rs_b, ALU.mult)
            out_writer(ct, sl, tmp)


def _emit_ln1_writes(tc, pools, pfx, io, mode="both"):
    """LN1 output writer: s8 (fp8, FFN1 input) and/or s16 (bf16 residual).

    mode "s8_pool" writes only s8, on the Pool engine (gelu-era DVE is
    saturated); the s16 pass is then recomputed later via mode "s16".
    """
    nc = tc.nc
    params = io["params"]
    if "s8" not in io:
        io["s8"] = pools["s_pool"].tile([P, CT, HW], FP8, tag=f"s8_{pfx}",
                                        name=f"s8_{pfx}")
        io["s16"] = pools["s_pool"].tile([P, CT, HW], BF16, tag=f"s16_{pfx}",
                                         name=f"s16_{pfx}")
    s8, s16 = io["s8"], io["s16"]

    def _ln1_write(ct, sl, tmp):
        if mode == "s16_to_pool":
            # s8 stays on DVE (critical path to the first gelus); s16
            # rides on Pool so av(7)-normalize starts earlier on DVE.
            nc.vector.tensor_scalar(
                out=s8[:, ct, sl], in0=tmp,
                scalar1=params["n1w"][:, ct:ct + 1],
                scalar2=params["n1b"][:, ct:ct + 1],
                op0=ALU.mult, op1=ALU.add,
            )
            nc.gpsimd.tensor_scalar(
                out=s16[:, ct, sl], in0=tmp,
                scalar1=params["n1w16"][:, ct:ct + 1],
                scalar2=params["n1b16"][:, ct:ct + 1],
                op0=ALU.mult, op1=ALU.add,
            )
            return
        if mode in ("both", "s8"):
            nc.vector.tensor_scalar(
                out=s8[:, ct, sl], in0=tmp,
                scalar1=params["n1w"][:, ct:ct + 1],
                scalar2=params["n1b"][:, ct:ct + 1],
                op0=ALU.mult, op1=ALU.add,
            )
        elif mode == "s8_pool":
            nc.gpsimd.tensor_scalar(
                out=s8[:, ct, sl], in0=tmp,
                scalar1=params["n1w"][:, ct:ct + 1],
                scalar2=params["n1b"][:, ct:ct + 1],
                op0=ALU.mult, op1=ALU.add,
            )
        if mode in ("both", "s16"):
            nc.vector.tensor_scalar(
                out=s16[:, ct, sl], in0=tmp,
                scalar1=params["n1w16"][:, ct:ct + 1],
                scalar2=params["n1b16"][:, ct:ct + 1],
                op0=ALU.mult, op1=ALU.add,
            )

    return _ln1_write


def _emit_ffn1(tc, pools, pfx, io, hts, after_ht=None):
    """FFN1 (fp8 DR, W1 x16) + gelu(scale=1/16) -> h fp8 [P, HT, HW].

    after_ht(ht) lets the caller drip other PE work (e.g. streamed FFN2
    accumulation over the just-finished h tiles) into the emission.
    """
    nc = tc.nc
    params = io["params"]
    w1 = io["w1"]
    if "h" not in io:
        io["h"] = pools["hbuf"].tile([P, HT, HW], FP8, tag="hbuf",
                                     name="hbuf")
    h = io["h"]
    s8 = io["s8"]
    for ht in hts:
        ph = pools["psum_s"].tile([P, HW], F32, tag="s", name="s")
        for ch in range(NCH):
            sl = slice(ch * 512, (ch + 1) * 512)
            for k in range(CT // 2):
                nc.tensor.matmul(
                    ph[:, sl],
                    lhsT=w1[:, 2 * k:2 * k + 2, ht * P:(ht + 1) * P],
                    rhs=s8[:, 2 * k:2 * k + 2, sl],
                    start=(k == 0), stop=(k == CT // 2 - 1),
                    perf_mode=DR,
                )
        nc.scalar.activation(
            out=h[:, ht, :], in_=ph, func=AF.Gelu,
            bias=params["b1"][:, ht:ht + 1], scale=1.0 / 16.0,
        )
        if after_ht is not None:
            after_ht(ht)


def _emit_ffn2_stream_start(tc, pools, io):
    """Held FFN2 ch0 accumulators for ct0/ct1 in the (idle) psum_av banks."""
    io["pfs"] = {
        ct: pools["psum_av"].tile([P, 512], F32, tag="av", name=f"pf{ct}")
        for ct in (0, 1)
    }


def _emit_ffn2_stream_k(tc, pools, io, k):
    """One k-pair of streamed FFN2 ch0 accumulation (needs h 2k,2k+1)."""
    nc = tc.nc
    w2, h = io["w2"], io["h"]
    for ct in (0, 1):
        nc.tensor.matmul(
            io["pfs"][ct],
            lhsT=w2[:, 2 * k:2 * k + 2, ct * P:(ct + 1) * P],
            rhs=h[:, 2 * k:2 * k + 2, 0:512],
            start=(k == 0), stop=(k == HT // 2 - 1),
            perf_mode=DR,
        )


def _emit_ffn2_stream_finish(tc, pools, pfx, io):
    """Evacuate the streamed ct0/ct1 ch0 psums -> r2 rows."""
    nc = tc.nc
    params = io["params"]
    if "r2" not in io:
        io["r2"] = pools["r_pool"].tile([P, CT, HW], BF16, tag=f"r_{pfx}",
                                        name=f"r2_{pfx}")
    for ct in (0, 1):
        nc.vector.scalar_tensor_tensor(
            out=io["r2"][:, ct, 0:512], in0=io["pfs"][ct],
            scalar=params["b216"][:, ct:ct + 1],
            in1=io["s16"][:, ct, 0:512], op0=ALU.add, op1=ALU.add,
        )
    del io["pfs"]


def _emit_ffn2(tc, pools, pfx, io, ct_chs):
    """FFN2 (fp8 DR, W2 x16) + 16*b2 + s16 residual -> r2_bf (=16*r2)."""
    nc = tc.nc
    params = io["params"]
    w2 = io["w2"]
    h = io["h"]
    psum_mm = pools["psum_mm"]
    if "r2" not in io:
        io["r2"] = pools["r_pool"].tile([P, CT, HW], BF16, tag=f"r_{pfx}",
                                        name=f"r2_{pfx}")
    r2_bf = io["r2"]
    for ct, ch in ct_chs:
        sl = slice(ch * 512, (ch + 1) * 512)
        pf = psum_mm.tile([P, 512], F32, tag="mm", name="mm")
        for k in range(HT // 2):
            nc.tensor.matmul(
                pf,
                lhsT=w2[:, 2 * k:2 * k + 2, ct * P:(ct + 1) * P],
                rhs=h[:, 2 * k:2 * k + 2, sl],
                start=(k == 0), stop=(k == HT // 2 - 1),
                perf_mode=DR,
            )
        nc.vector.scalar_tensor_tensor(
            out=r2_bf[:, ct, sl], in0=pf, scalar=params["b216"][:, ct:ct + 1],
            in1=io["s16"][:, ct, sl], op0=ALU.add, op1=ALU.add,
        )


def _emit_ln2_norm(tc, pools, pfx, io, ch, act_scale=False):
    """LN2 normalize+scale+DMA out for one chunk.

    act_scale=True puts the *w+b step on ACT (Identity, table-free) --
    only for tail chunks where ACT is otherwise idle.
    """
    nc = tc.nc
    params, out_dram = io["params"], io["out"]
    st = io["ln2st"]

    def _ln2_write(ct, sl, tmp):
        o32 = pools["ostage"].tile([P, 512], F32, tag="ostage", name="ostage",
                                   bufs=2)
        if act_scale:
            nc.scalar.activation(
                out=o32, in_=tmp, func=AF.Identity,
                bias=params["n2b"][:, ct:ct + 1],
                scale=params["n2w"][:, ct:ct + 1],
            )
        else:
            nc.vector.tensor_scalar(
                out=o32, in0=tmp,
                scalar1=params["n2w"][:, ct:ct + 1],
                scalar2=params["n2b"][:, ct:ct + 1],
                op0=ALU.mult, op1=ALU.add,
            )
        eng = (nc.sync, nc.gpsimd, nc.scalar, nc.sync)[ct % 4]
        eng.dma_start(out=out_dram[ct * P:(ct + 1) * P, sl], in_=o32)

    _emit_ln_norm(tc, pools, io["r2"], st, _ln2_write, chunks=(ch,))


def _emit_ln2_stats(tc, pools, pfx, io, ch, sq_dve=True, rsqrt=True):
    st = io.setdefault("ln2st", {})
    _emit_ln_stats(tc, pools, io["r2"], st, io["inv512"], io["eps"],
                   chunks=(ch,), sq_dve=sq_dve, rsqrt=rsqrt)


def build_program():
    nc = bacc.Bacc("TRN2", target_bir_lowering=False, debug=False)

    def din(name, shape, dt):
        return nc.dram_tensor(name, list(shape), dt, kind="ExternalInput").ap()

    x8 = {p: din(f"x_{p}8", (P, CT, HW), FP8) for p in "sf"}
    x256 = {p: din(f"x_{p}256", (P, CT, HW), BF16) for p in "sf"}
    wq8 = {p: din(f"{p}_wq8", (P, CT, C), FP8) for p in "sf"}
    wk8 = {p: din(f"{p}_wk8", (P, CT, C), FP8) for p in "sf"}
    wv8 = {p: din(f"{p}_wv8", (P, CT, C), FP8) for p in "sf"}
    wo8 = {p: din(f"{p}_wo8", (P, CT, C), FP8) for p in "sf"}
    w18 = {p: din(f"{p}_w18", (P, CT, HID), FP8) for p in "sf"}
    w28 = {p: din(f"{p}_w28", (P, HT, C), FP8) for p in "sf"}
    prm_d = din("prm", (P, 2, NPCOL), F32)
    outs = {
        p: nc.dram_tensor(f"out_{p}", [C, HW], F32, kind="ExternalOutput").ap()
        for p in "sf"
    }

    with tile.TileContext(nc) as tc:
        from contextlib import ExitStack
        with ExitStack() as ctx:
            pools = {}

            def pool(name, bufs, space="SBUF", stack=None):
                pools[name] = (stack or ctx).enter_context(
                    tc.tile_pool(name=name, bufs=bufs, space=space)
                )
                return pools[name]

            # whole-program pools
            pool("psum_mm", 2, space="PSUM")
            pool("psum_s", 2, space="PSUM")
            pool("psum_av", 2, space="PSUM")
            pool("consts", 1)
            pool("params", 1)
            pool("rows", 1)
            pool("xr", 2)
            pool("bcast", 1)
            pool("tmp", 1)
            pool("sq", 2)
            pool("rz", 1)
            pool("pt", 3)
            pool("qkv", 1)
            pool("o_pool", 1)
            pool("wo_pool", 1)
            pool("r_pool", 1)
            pool("s_pool", 1)
            pool("ostage", 2)
            # x8 + wproj live only through the projections; their SBUF is
            # reused by x256/wffn/hbuf afterwards (LIFO close below).
            xw_stack = ctx.enter_context(ExitStack())
            pool("x8", 1, stack=xw_stack)
            pool("wproj", 2, stack=xw_stack)

            inv512 = pools["consts"].tile([P, 1], BF16)
            nc.vector.memset(inv512, 1.0 / C)
            eps_sb = pools["consts"].tile([1, 1], F32)
            nc.vector.memset(eps_sb, EPS)
            # Pin the ACT table set to natural_log_exp_and_others (Ln+Exp)
            # before the softmax exps start.
            lnpin = pools["consts"].tile([1, 1], F32)
            nc.vector.memset(lnpin, 1.0)
            nc.scalar.activation(lnpin, lnpin, AF.Ln, bias=eps_sb[:, 0:1])

            # PE warm-up: junk matmuls spend the p-state ramp (~3us to full
            # clock) while the first DMAs land, so the real projections run
            # at full speed. Nothing reads the psum tiles.
            warm = pools["consts"].tile([P, 2, 512], FP8)
            nc.vector.memset(warm, 0.0625)
            for _ in range(22):
                pw = pools["psum_mm"].tile([P, 512], F32, tag="mm",
                                           name="warm")
                nc.tensor.matmul(pw, lhsT=warm[:, :, 0:P], rhs=warm,
                                 start=True, stop=True, perf_mode=DR)

            # ---- bulk loads: one contiguous DMA each, rotating queues ----
            _q = [nc.sync, nc.gpsimd, nc.scalar]
            _qi = [0]

            def dma_next(out, in_):
                eng = _q[_qi[0] % len(_q)]
                _qi[0] += 1
                eng.dma_start(out=out, in_=in_)

            x8_sb = {}
            w_sb = {}
            for p in "sf":
                x8_sb[p] = pools["x8"].tile([P, CT, HW], FP8, tag=f"x8_{p}",
                                            name=f"x8_{p}")
            for nm, src in (("wq_s", wq8["s"]), ("wk_s", wk8["s"])):
                w_sb[nm] = pools["wproj"].tile([P, CT, C], FP8, tag=nm[:2],
                                               name=nm)
            # head-critical loads first: x_s, wq_s, x_f, wk_s
            dma_next(x8_sb["s"], x8["s"])
            dma_next(w_sb["wq_s"], wq8["s"])
            dma_next(x8_sb["f"], x8["f"])
            dma_next(w_sb["wk_s"], wk8["s"])

            # params block (single small DMA)
            prm_sb = pools["params"].tile([P, 2, NPCOL], F32, tag="prm")
            dma_next(prm_sb, prm_d)
            params = {}
            for pi, p in enumerate("sf"):
                params[p] = {}
                for i, n in enumerate(PNAMES):
                    params[p][n] = prm_sb[:, pi, i * CT:(i + 1) * CT]
                params[p]["b1"] = prm_sb[:, pi, len(PNAMES) * CT:NPCOL]

            qkv = {}
            for p in "sf":
                for g in range(2):
                    qkv[f"q_{p}{g}"] = pools["qkv"].tile(
                        [P, 2, HW], FP8, tag=f"q_{p}{g}", name=f"q_{p}{g}")
                    qkv[f"k_{p}{g}"] = pools["qkv"].tile(
                        [P, 2, HW], FP8, tag=f"k_{p}{g}", name=f"k_{p}{g}")
                qkv[f"v_{p}"] = pools["qkv"].tile(
                    [P, TT, HEADS, VW], FP8, tag=f"v_{p}", name=f"v_{p}")

            wo_sb = {}
            o_sb = {}
            for p in "sf":
                wo_sb[p] = pools["wo_pool"].tile([P, CT, C], FP8,
                                                 tag=f"wo_{p}", name=f"wo_{p}")
                o_sb[p] = pools["o_pool"].tile([P, HEADS // 2, HW], FP8,
                                               tag=f"o_{p}", name=f"o_{p}")

            ios = {}
            for p in "sf":
                ios[p] = {
                    "o": o_sb[p], "wo": wo_sb[p],
                    "params": params[p], "out": outs[p],
                    "inv512": inv512, "eps": eps_sb,
                }

            # software-pipelined attention: S^T+exp of pair N overlaps
            # AV of pair N-1 on PE, so PE never waits on the ACT exp chain.
            # stream 's': q from x_s, kv from x_f ; stream 'f': swapped
            seq = [("s", hp) for hp in range(4)] + [("f", hp) for hp in range(4)]
            pts = {}

            def st(i, filler=None):
                p, hp = seq[i]
                g = hp // 2
                pts[i] = _emit_st_exp(tc, pools, hp, qkv[f"q_{p}{g}"],
                                      qkv[f"k_{p}{g}"], filler=filler)

            def av(i):
                p, hp = seq[i]
                _emit_av(tc, pools, hp, pts.pop(i), qkv[f"v_{p}"], o_sb[p])

            qs2 = [qkv["q_s0"], qkv["q_s1"]]
            ks2 = [qkv["k_s0"], qkv["k_s1"]]
            qf2 = [qkv["q_f0"], qkv["q_f1"]]
            kf2 = [qkv["k_f0"], qkv["k_f1"]]

            # ---- A(s): group g0 first so st(0)/st(1) start early ----
            _emit_proj_qk_one(tc, pools, x8_sb["s"], w_sb["wq_s"], qs2,
                              ts=(0, 1), act_evict=True)
            _emit_proj_qk_one(tc, pools, x8_sb["f"], w_sb["wk_s"], ks2,
                              ts=(0, 1), act_evict=True)
            st(0)
            _emit_proj_qk_one(tc, pools, x8_sb["s"], w_sb["wq_s"], qs2,
                              ts=(2, 3))
            _emit_proj_qk_one(tc, pools, x8_sb["f"], w_sb["wk_s"], ks2,
                              ts=(2, 3))
            st(1)
            # next round of loads
            w_sb["wv_s"] = pools["wproj"].tile([P, CT, C], FP8, tag="wv",
                                               name="wv_s")
            w_sb["wq_f"] = pools["wproj"].tile([P, CT, C], FP8, tag="wq",
                                               name="wq_f")
            w_sb["wk_f"] = pools["wproj"].tile([P, CT, C], FP8, tag="wk",
                                               name="wk_f")
            w_sb["wv_f"] = pools["wproj"].tile([P, CT, C], FP8, tag="wv",
                                               name="wv_f")
            dma_next(w_sb["wv_s"], wv8["s"])
            dma_next(w_sb["wq_f"], wq8["f"])
            dma_next(w_sb["wk_f"], wk8["f"])
            nc.vector.memset(qkv["v_s"][:, :, :, DH + 1:], 0.0)
            _emit_proj_v(tc, pools, x8_sb["f"], w_sb["wv_s"], qkv["v_s"])
            av(0)
            st(2)
            dma_next(w_sb["wv_f"], wv8["f"])
            dma_next(wo_sb["s"], wo8["s"])
            dma_next(wo_sb["f"], wo8["f"])
            av(1)
            _emit_proj_qk_one(tc, pools, x8_sb["f"], w_sb["wq_f"], qf2,
                              ts=(0, 1))
            _emit_proj_qk_one(tc, pools, x8_sb["s"], w_sb["wk_f"], kf2,
                              ts=(0, 1))
            st(3)
            av(2)
            _emit_proj_qk_one(tc, pools, x8_sb["f"], w_sb["wq_f"], qf2,
                              ts=(2, 3))
            _emit_proj_qk_one(tc, pools, x8_sb["s"], w_sb["wk_f"], kf2,
                              ts=(2, 3))
            nc.vector.memset(qkv["v_f"][:, :, :, DH + 1:], 0.0)
            _emit_proj_v(tc, pools, x8_sb["s"], w_sb["wv_f"], qkv["v_f"])
            # projections done: free x8/wproj SBUF for wffn/hbuf
            xw_stack.close()
            pool("wffn", 1)
            pool("hbuf", 1)
            for p in "sf":
                ios[p]["x256d"] = x256[p]
            ios["s"]["w1"] = pools["wffn"].tile([P, CT, HID], FP8, tag="w1",
                                                name="w1_s", bufs=1)
            ios["s"]["w2"] = pools["wffn"].tile([P, HT, C], FP8, tag="w2",
                                                name="w2_s", bufs=2)
            dma_next(ios["s"]["w1"], w18["s"])
            dma_next(ios["s"]["w2"], w28["s"])
            # av/wo/LN1(s) ride as fillers INSIDE the st blocks: their PE
            # work lands in the PE slack of the block instead of delaying
            # the next block's S^T matmuls (which would starve the exps).
            st(4)
            av(3)
            st(5)
            _emit_wo_residual(tc, pools, "s", ios["s"], (0, 1))
            av(4)
            st(6)
            _emit_wo_residual(tc, pools, "s", ios["s"], (2, 3))
            av(5)
            # LN1(s) fully inside the attention era: rsqrt shares the
            # softmax Ln+Exp table set, the normalize rides the idle DVE
            # slack, so s8 is ready the moment the last exp retires.
            ios["s"]["ln1st"] = {}
            _emit_ln_stats(tc, pools, ios["s"]["r"], ios["s"]["ln1st"],
                           inv512, eps_sb, rsqrt=True)
            st(7)
            av(6)
            _emit_ln_norm(tc, pools, ios["s"]["r"], ios["s"]["ln1st"],
                          _emit_ln1_writes(tc, pools, "s", ios["s"]))
            av(7)

            # ---- gelu era. ACT order: gelus(s), ln1(f), gelus(f),
            # ln2(s,0), ln2(s,1), ln2(f,0), ln2(f,1) -> 4 table reloads.
            # FFN2 ch0 ct0/ct1 accumulate in the idle psum_av banks DURING
            # the gelus (k-pair k right after gelu 2k+1), so only ct2/ct3
            # remain after the last gelu. gelus(s) start right after the
            # last exp; wo_residual(f) runs on PE behind the FFN1(s) mms.
            # DVE is the scarce engine during gelus(s): LN1(f) writes only
            # s8 there (TSPs on Pool); the s16 pass is recomputed during
            # gelus(f) when DVE is light again.
            _emit_ffn2_stream_start(tc, pools, ios["s"])
            ios["f"]["ln1st"] = {}

            def s_after_ht(ht):
                # Only drip PE work whose deps are certainly met at this
                # queue position (PE is in-order: a stalled drip matmul
                # blocks the FFN1 matmuls and starves the gelus).
                if ht % 2 == 1:
                    _emit_ffn2_stream_k(tc, pools, ios["s"], ht // 2)

            _emit_ffn1(tc, pools, "s", ios["s"], range(0, 16),
                       after_ht=s_after_ht)
            _emit_wo_residual(tc, pools, "f", ios["f"], (0, 1))
            _emit_wo_residual(tc, pools, "f", ios["f"], (2, 3))
            _emit_ln_stats(tc, pools, ios["f"]["r"], ios["f"]["ln1st"],
                           inv512, eps_sb, rsqrt=True)
            _emit_ln_norm(tc, pools, ios["f"]["r"], ios["f"]["ln1st"],
                          _emit_ln1_writes(tc, pools, "f", ios["f"],
                                           mode="s8_pool"))
            _emit_ffn2_stream_finish(tc, pools, "s", ios["s"])
            _emit_ffn2(tc, pools, "s", ios["s"], [(2, 0), (3, 0)])
            _emit_ln2_stats(tc, pools, "s", ios["s"], 0, rsqrt=False)
            ios["f"]["w1"] = pools["wffn"].tile([P, CT, HID], FP8, tag="w1",
                                                name="w1_f", bufs=1)
            dma_next(ios["f"]["w1"], w18["f"])
            ios["f"]["w2"] = pools["wffn"].tile([P, HT, C], FP8, tag="w2",
                                                name="w2_f", bufs=2)
            dma_next(ios["f"]["w2"], w28["f"])
            # f-stream gelus; FFN2(f) ch0 streams into psum_av, and the
            # four FFN2(s) ch1 column blocks drip in behind h_s (complete).
            _emit_ffn2_stream_start(tc, pools, ios["f"])
            # s16(f) is only needed by the FFN2(f) stream-finish stts;
            # recomputing it here (gelus(f) window) keeps it off the
            # saturated gelus(s)-era DVE.
            _emit_ln_norm(tc, pools, ios["f"]["r"], ios["f"]["ln1st"],
                          _emit_ln1_writes(tc, pools, "f", ios["f"],
                                           mode="s16"))

            def f_after_ht(ht):
                if ht % 2 == 1:
                    _emit_ffn2_stream_k(tc, pools, ios["f"], ht // 2)
                    if ht <= 7:
                        _emit_ffn2(tc, pools, "s", ios["s"],
                                   [((ht - 1) // 2, 1)])

            _emit_ffn1(tc, pools, "f", ios["f"], range(0, 16),
                       after_ht=f_after_ht)
            _emit_ln_rsqrt(tc, pools, ios["s"]["ln2st"], eps_sb, chunks=(0,))
            _emit_ln2_norm(tc, pools, "s", ios["s"], 0)
            _emit_ln2_stats(tc, pools, "s", ios["s"], 1)
            _emit_ln2_norm(tc, pools, "s", ios["s"], 1)
            _emit_ffn2_stream_finish(tc, pools, "f", ios["f"])
            _emit_ffn2(tc, pools, "f", ios["f"], [(2, 0), (3, 0)])
            _emit_ln2_stats(tc, pools, "f", ios["f"], 0)
            _emit_ln2_norm(tc, pools, "f", ios["f"], 0, act_scale=True)
            _emit_ffn2(tc, pools, "f", ios["f"],
                       [(ct, 1) for ct in range(CT)])
            _emit_ln2_stats(tc, pools, "f", ios["f"], 1)
            _emit_ln2_norm(tc, pools, "f", ios["f"], 1, act_scale=True)

    nc.compile()
    return nc


# --------------------------------------------------------------------------
# host side
# --------------------------------------------------------------------------

_BF = ml_dtypes.bfloat16
_F8 = ml_dtypes.float8_e4m3
WS = 16.0  # host weight scale for fp8 matmuls
XS = WS * WS  # attention product scale absorbed by LN1


def _head_perm():
    """Permuted output-channel order for Q/K projections.

    Tile t = 2g+hi, partition 32*h4+lo  ->  orig channel (4g+h4)*64+32*hi+lo.
    """
    perm = np.zeros(C, dtype=np.int64)
    i = 0
    for g in range(2):
        for hi in range(2):
            for h4 in range(4):
                for lo in range(32):
                    perm[i] = (4 * g + h4) * 64 + 32 * hi + lo
                    i += 1
    return perm


def _pmajor(m, tiles):
    """[tiles*P, X] -> [P, tiles, X] with row c = t*P + p."""
    return np.ascontiguousarray(
        m.reshape(tiles, P, m.shape[1]).transpose(1, 0, 2)
    )


def _prep_shared_inputs(inputs):
    """Host-side weight prep: transposes, casts, permutations, x16 scales."""
    sh = {}
    perm = _head_perm()
    prm = np.zeros((P, 2, NPCOL), np.float32)
    for pi, (p, ap) in enumerate((("s", "s_"), ("f", "f_"))):
        wq, wk, wv, wo = (inputs[ap + n] for n in ("Wq", "Wk", "Wv", "Wo"))
        sh[f"{p}_wq8"] = _pmajor((wq.T[:, perm] * WS).astype(_F8), CT)
        sh[f"{p}_wk8"] = _pmajor((wk.T[:, perm] * WS).astype(_F8), CT)
        sh[f"{p}_wv8"] = _pmajor((wv.T * WS).astype(_F8), CT)
        sh[f"{p}_wo8"] = _pmajor((wo.T * WS).astype(_F8), CT)
        w1 = inputs[f"{p}ffn_W1"]
        w2 = inputs[f"{p}ffn_W2"]
        sh[f"{p}_w18"] = _pmajor((w1.T * WS).astype(_F8), CT)
        sh[f"{p}_w28"] = _pmajor((w2.T * WS).astype(_F8), HT)
        vals = {
            "bo256": inputs[ap + "bo"] * XS,
            "n1w": inputs[f"{p}n1_w"], "n1b": inputs[f"{p}n1_b"],
            "n1w16": inputs[f"{p}n1_w"] * WS,
            "n1b16": inputs[f"{p}n1_b"] * WS,
            "n2w": inputs[f"{p}n2_w"], "n2b": inputs[f"{p}n2_b"],
            "b216": inputs[f"{p}ffn_b2"] * WS,
        }
        for i, n in enumerate(PNAMES):
            prm[:, pi, i * CT:(i + 1) * CT] = vals[n].reshape(CT, P).T
        prm[:, pi, len(PNAMES) * CT:NPCOL] = (
            inputs[f"{p}ffn_b1"].reshape(HT, P).T
        )
    sh["prm"] = prm
    return sh


def make_in_maps(inputs):
    shared = _prep_shared_inputs(inputs)
    xs = np.ascontiguousarray(inputs["spatial_feat"].reshape(B, C, HW))
    xf = np.ascontiguousarray(inputs["freq_feat"].reshape(B, C, HW))
    in_maps = []
    for b in range(N_CORES):
        m = dict(shared)
        m["x_s8"] = _pmajor(xs[b].astype(_F8), CT)
        m["x_f8"] = _pmajor(xf[b].astype(_F8), CT)
        m["x_s256"] = _pmajor((xs[b] * XS).astype(_BF), CT)
        m["x_f256"] = _pmajor((xf[b] * XS).astype(_BF), CT)
        in_maps.append(m)
    return in_maps


_CACHED = {}


def _get_program():
    if "nc" not in _CACHED:
        _CACHED["nc"] = build_program()
    return _CACHED["nc"]


def run_on_hw(inputs, trace=False, trace_kwargs=None):
    from concourse.bass_utils import run_bass_kernel_spmd

    nc = _get_program()
    in_maps = make_in_maps(inputs)
    res = run_bass_kernel_spmd(
        nc, in_maps, list(range(N_CORES)), trace=trace,
        **(dict(trace_kwargs=trace_kwargs) if trace_kwargs else {}),
    )
    s = np.stack([res.results[b]["out_s"] for b in range(B)])
    f = np.stack([res.results[b]["out_f"] for b in range(B)])
    s = s.reshape(B, C, H_IMG, W_IMG).astype(np.float32)
    f = f.reshape(B, C, H_IMG, W_IMG).astype(np.float32)
    return (s, f), res


def kernel(**inputs):
    out, _ = run_on_hw(inputs, trace=False)
    return out


if __name__ == "__main__":
    import reference

    inputs = {k: np.asarray(v) for k, v in reference.setup_inputs().items()}
    exp_s, exp_f = reference.reference(**inputs)
    act_s, act_f = kernel(**inputs)
    for nm, e, a in (("s", exp_s, act_s), ("f", exp_f, act_f)):
        err = np.abs(np.asarray(a) - np.asarray(e)).max()
        print(nm, "absmax", err, "rel", err / np.abs(e).max())


# revision 41
# speedup vs baseline: 1.0011x; 1.0002x over previous
"""CrossFusionBlock Trainium2 kernel.

Dual-stream cross-attention block (B=8, C=512, HW=1024, 8 heads, FFN 2048).
Sharding: data-parallel over batch across 8 NeuronCores (1 batch element per
core), weights replicated. All weight transposes / casts / permutations are
done on the host so the device kernel contains no transposes at all.

v2: ACT (scalar engine) is the bottleneck (softmax exp 131k lanes-cycles +
gelu). Everything else is scheduled around keeping ACT saturated:
  - all projections fp8 DoubleRow (matmul cost = out-cols only, fp8 DR
    halves it); proj weights host-scaled x16 so fp8 e4m3 is well covered.
    Attention product scale 16*16=256 absorbed by LN1 (x256 residual).
  - p-major host layouts: every weight/activation load is one contiguous
    DMA; params ride in a single [P,2,48] block. First exp at ~5us.
  - softmax normalize: reciprocal writes partition 0 (misaligned DVE op),
    Pool partition_broadcast fans out 1/Z, both head-halves written by DVE
    tensor ops (par1 with misaligned out partitions) -- no DMAs at all.
  - LN mu/rs broadcasts on Pool partition_broadcast instead of DMA.
  - ACT order: [exps | LN1(s) rsqrt inline (shared Ln+Exp table set),
    gelus(s), LN1(f) rsqrt, gelus(f), LN2(s+f) rsqrt] -> 4 table reloads,
    no ACT stalls. LN1(f) stats emitted early, its ACT part late.
  - FFN2/LN2 chunk-interleaved so the tail chain is one half-stream deep.
"""

import sys

import numpy as np

for _p in ("/opt/trn_rl_repo", "/opt/pypackages"):
    if _p not in sys.path:
        sys.path.insert(0, _p)

import ml_dtypes  # noqa: E402

import concourse.bass as bass  # noqa: E402
from concourse import bacc  # noqa: E402
import concourse.mybir as mybir  # noqa: E402
import concourse.tile as tile  # noqa: E402


def _patch_act_tables():
    """Make natural_log_exp_and_others the only set offering Exp/Ln.

    The table-load pass greedily picks the first set containing each
    activation function, which ping-pongs between the ln-only and exp-only
    sets (2 table loads per LayerNorm rsqrt). Hiding Exp/Ln from the other
    sets forces the combined set; set ids keep their true act_info indices
    so the emitted LoadActFuncSet ids stay valid for walrus.
    """
    import concourse.hw_specs as hw_specs

    if getattr(hw_specs, "_act_tables_patched", False):
        return
    orig = hw_specs.get_activation_tables

    def patched(arch):
        tabs = dict(orig(arch))
        exp = mybir.ActivationFunctionType.Exp
        ln = mybir.ActivationFunctionType.Ln
        out = {}
        for name, funcs in tabs.items():
            if name != "natural_log_exp_and_others":
                funcs = funcs - {exp, ln}
            out[name] = funcs
        return out

    hw_specs._act_tables_patched = True
    hw_specs.get_activation_tables = patched
    bacc.get_activation_tables = patched


_patch_act_tables()

P = 128
C = 512
HW = 1024
HEADS = 8
DH = 64
HID = 2048
CT = C // P        # 4 channel tiles
HT = HID // P      # 16 hidden tiles
TT = HW // P       # 8 token tiles
NCH = HW // 512    # 2 free-dim chunks of 512
EPS = 1e-6
BF16 = mybir.dt.bfloat16
FP8 = mybir.dt.float8e4
F32 = mybir.dt.float32
AF = mybir.ActivationFunctionType
ALU = mybir.AluOpType
DR = mybir.MatmulPerfMode.DoubleRow

N_CORES = 8
B, H_IMG, W_IMG = 8, 32, 32

VW = 72  # V row width: DH + ones col + zero pad (16B-aligned for DoubleRow)

# param block column layout: 8 x [P, CT] then b1 [P, HT]
PNAMES = ("bo256", "n1w", "n1b", "n1w16", "n1b16", "n2w", "n2b", "b216")
NPCOL = len(PNAMES) * CT + HT  # 48


# --------------------------------------------------------------------------
# device program
# --------------------------------------------------------------------------

def _emit_proj_qk_one(tc, pools, x8, w8, out2, ts=tuple(range(CT)),
                      act_evict=False):
    """fp8 DR projection with head-grouped permuted W -> out2 = [g0, g1].

    Psum tile t = 2*g + hi holds rows (head 4g+h4, d = 32*hi + lo) at
    partition 32*h4 + lo; evacuated to out2[g][:, hi, :]. act_evict
    alternates psum eviction between ACT and DVE -- only useful in the
    head, while ACT still has no exps to chew on.
    """
    nc = tc.nc
    psum_mm = pools["psum_mm"]
    i = 0
    for t in ts:
        g, hi = t // 2, t % 2
        for ch in range(NCH):
            pq = psum_mm.tile([P, 512], F32, tag="mm", name="mm")
            for k in range(CT // 2):
                nc.tensor.matmul(
                    pq,
                    lhsT=w8[:, 2 * k:2 * k + 2, t * P:(t + 1) * P],
                    rhs=x8[:, 2 * k:2 * k + 2, ch * 512:(ch + 1) * 512],
                    start=(k == 0), stop=(k == CT // 2 - 1),
                    perf_mode=DR,
                )
            dst = out2[g][:, hi, ch * 512:(ch + 1) * 512]
            if act_evict and i % 2 == 0:
                nc.scalar.copy(out=dst, in_=pq)
            else:
                nc.vector.tensor_copy(out=dst, in_=pq)
            i += 1


def _emit_proj_v(tc, pools, xkv8, wv8, v_hf):
    nc = tc.nc
    psum_mm = pools["psum_mm"]
    for tt in range(TT):
        pv = psum_mm.tile([P, 512], F32, tag="mm", name="mm")
        for k in range(CT // 2):
            nc.tensor.matmul(
                pv,
                lhsT=xkv8[:, 2 * k:2 * k + 2, tt * P:(tt + 1) * P],
                rhs=wv8[:, 2 * k:2 * k + 2, :],
                start=(k == 0), stop=(k == CT // 2 - 1),
                perf_mode=DR,
            )
        nc.vector.tensor_copy(
            out=v_hf[:, tt, :, 0:DH],
            in_=pv.rearrange("p (h d) -> p h d", d=DH),
        )
        nc.vector.memset(v_hf[:, tt, :, DH:DH + 1], 1.0)


def _emit_st_exp(tc, pools, hp, q2g, k2g, filler=None):
    """S^T per head via fp8 DoubleRow (Ki=32 x2) -> exp(P^T*2^-11) fp8.

    filler() is emitted after tt==1: its PE work rides in the huge PE
    slack inside the st block instead of BETWEEN st blocks, where it
    would delay the next block's S^T matmuls and starve the exp stream.
    """
    nc = tc.nc
    pt = {}
    for par in (0, 1):
        pt[par] = pools["pt"].tile([P, TT, HW], FP8, tag="pt", name="pt",
                                   bufs=4)
    for tt in range(TT):
        if tt == 2 and filler is not None:
            filler()
        ps = {}
        for par in (0, 1):
            h4 = (2 * hp + par) % 4
            base = 32 * h4
            kw = {"tile_position": (96, 0)} if h4 == 3 else {}
            p_s = pools["psum_s"].tile([P, HW], F32, tag="s", name="s")
            for ch in range(NCH):
                nc.tensor.matmul(
                    p_s[:, ch * 512:(ch + 1) * 512],
                    lhsT=k2g[base:base + 32, :, tt * P:(tt + 1) * P],
                    rhs=q2g[base:base + 32, :, ch * 512:(ch + 1) * 512],
                    start=True, stop=True,
                    perf_mode=DR,
                    **kw,
                )
            ps[par] = p_s
        for par in (0, 1):
            nc.scalar.activation(out=pt[par][:, tt, :], in_=ps[par],
                                 func=AF.Exp, scale=0.125 / 256.0)
    return pt


def _emit_av(tc, pools, hp, pt, v_hf, o_pair):
    """AV+Z (ones column) fp8 DR -> normalize into o_pair[:, hp].

    Z sits at psum row DH; its reciprocal is written to partition 0 of a
    [1,512] tile (misaligned DVE op), Pool broadcasts it to 64 partitions,
    then both head-halves are normalized by DVE tensor ops (par1 writes
    partitions 64:128 from psum rows 0:64 -- misaligned out).
    """
    nc = tc.nc
    for par in (0, 1):
        h = 2 * hp + par
        for ch in range(NCH):
            sl = slice(ch * 512, (ch + 1) * 512)
            pav = pools["psum_av"].tile([P, 512], F32, tag="av", name="av")
            for tt2 in range(TT // 2):
                nc.tensor.matmul(
                    pav[0:VW, :],
                    lhsT=v_hf[:, 2 * tt2:2 * tt2 + 2, h, :],
                    rhs=pt[par][:, 2 * tt2:2 * tt2 + 2, sl],
                    start=(tt2 == 0), stop=(tt2 == TT // 2 - 1),
                    perf_mode=DR,
                )
            rz0 = pools["rz"].tile([1, 512], BF16, tag="rz0", name="rz0",
                                   bufs=4)
            with nc.allow_low_precision(reason="1/Z feeds an fp8 store"):
                nc.vector.reciprocal(out=rz0, in_=pav[DH:DH + 1, :])
            rzb = pools["rz"].tile([DH, 512], BF16, tag="rzb", name="rzb",
                                   bufs=4)
            nc.gpsimd.partition_broadcast(rzb, rz0, channels=DH)
            nc.vector.tensor_tensor(
                o_pair[par * DH:(par + 1) * DH, hp, sl],
                pav[0:DH, :], rzb, ALU.mult,
            )


def _emit_wo_residual(tc, pools, pfx, io, cts):
    """Wo(16x fp8) projection + 256*bo + 256*x residual -> r_bf (=256*r)."""
    nc = tc.nc
    o_hf = io["o"]
    wo, params = io["wo"], io["params"]
    psum_mm = pools["psum_mm"]
    if "r" not in io:
        io["r"] = pools["r_pool"].tile([P, CT, HW], BF16, tag=f"r_{pfx}",
                                       name=f"r_{pfx}")
    r_bf = io["r"]
    for ct in cts:
        x256 = pools["xr"].tile([P, HW], BF16, tag="xr", name="xr", bufs=2)
        nc.gpsimd.dma_start(out=x256, in_=io["x256d"][:, ct, :])
        for ch in range(NCH):
            sl = slice(ch * 512, (ch + 1) * 512)
            pe_ = psum_mm.tile([P, 512], F32, tag="mm", name="mm")
            for i2 in range(HEADS // 4):
                nc.tensor.matmul(
                    pe_,
                    lhsT=wo[:, 2 * i2:2 * i2 + 2, ct * P:(ct + 1) * P],
                    rhs=o_hf[:, 2 * i2:2 * i2 + 2, sl],
                    start=(i2 == 0), stop=(i2 == HEADS // 4 - 1),
                    perf_mode=DR,
                )
            nc.vector.scalar_tensor_tensor(
                out=r_bf[:, ct, sl], in0=pe_,
                scalar=params["bo256"][:, ct:ct + 1],
                in1=x256[:, sl], op0=ALU.add, op1=ALU.add,
            )


def _emit_ln_stats(tc, pools, src_bf, st, inv512, eps_sb,
                   chunks=tuple(range(NCH)), sq_dve=False, rsqrt=True):
    """LN stats over the channel (partition x 4-tile) axis of [P, CT, HW].

    Fills st["mu2"]/st["rs2"] rows ([1, NCH, 512]). rsqrt=True also emits
    the ACT Ln+Exp pair per chunk; rsqrt=False defers it to
    _emit_ln_rsqrt (so the ACT ops can be queued later).
    """
    nc = tc.nc
    psum_mm = pools["psum_mm"]
    if "mu2" not in st:
        st["mu2"] = pools["rows"].tile([1, NCH, 512], BF16, tag="mu2",
                                       name="mu2", bufs=2)
        st["var2"] = pools["rows"].tile([1, NCH, 512], F32, tag="var2",
                                        name="var2", bufs=2)
        st["rs2"] = pools["rows"].tile([1, NCH, 512], BF16, tag="rs2",
                                       name="rs2", bufs=2)
    mu2, var2, rs2 = st["mu2"], st["var2"], st["rs2"]
    for ch in chunks:
        sl = slice(ch * 512, (ch + 1) * 512)
        pmu = psum_mm.tile([1, 512], F32, tag="mm", name="mm")
        for k in range(CT):
            nc.tensor.matmul(
                pmu, lhsT=inv512[:, 0:1], rhs=src_bf[:, k, sl],
                start=(k == 0), stop=(k == CT - 1),
            )
        pms = psum_mm.tile([1, 512], F32, tag="mm", name="mm")
        for k in range(CT):
            r2 = pools["sq"].tile([P, 512], BF16, tag="sq", name="sq")
            if sq_dve == "alt":
                sq_eng = nc.vector if k % 2 else nc.gpsimd
            else:
                sq_eng = nc.vector if sq_dve else nc.gpsimd
            sq_eng.tensor_tensor(r2, src_bf[:, k, sl], src_bf[:, k, sl],
                                 ALU.mult)
            nc.tensor.matmul(
                pms, lhsT=inv512[:, 0:1], rhs=r2,
                start=(k == 0), stop=(k == CT - 1),
            )
        nc.vector.tensor_copy(out=mu2[0:1, ch, :], in_=pmu)
        musq = pools["rows"].tile([1, 512], F32, tag="musq", name="musq",
                                  bufs=2)
        nc.vector.tensor_tensor(musq, mu2[0:1, ch, :], mu2[0:1, ch, :],
                                ALU.mult)
        nc.vector.tensor_tensor(var2[0:1, ch, :], pms, musq, ALU.subtract)
        if rsqrt:
            _emit_ln_rsqrt(tc, pools, st, eps_sb, chunks=(ch,))


def _emit_ln_rsqrt(tc, pools, st, eps_sb, chunks=tuple(range(NCH))):
    """rs = 1/sqrt(var+eps) = exp(-0.5*ln(var+eps)) on ACT (Ln+Exp set)."""
    nc = tc.nc
    for ch in chunks:
        lnv = pools["rows"].tile([1, 512], F32, tag="lnv", name="lnv", bufs=2)
        nc.scalar.activation(lnv, st["var2"][0:1, ch, :], AF.Ln,
                             bias=eps_sb[:, 0:1])
        nc.scalar.activation(st["rs2"][0:1, ch, :], lnv, AF.Exp, scale=-0.5)


def _emit_ln_norm(tc, pools, src_bf, st, out_writer,
                  chunks=tuple(range(NCH)), pool_cts=()):
    """Broadcast mu/rs (Pool) and hand normalized [P,512] bf16 pieces on.

    pool_cts: ct indices whose subtract/mult pair runs on Pool instead of
    DVE (load-balancing for windows where DVE is saturated).
    """
    nc = tc.nc
    for ch in chunks:
        sl = slice(ch * 512, (ch + 1) * 512)
        mu_b = pools["bcast"].tile([P, 512], BF16, tag="mu_b", name="mu_b",
                                   bufs=2)
        rs_b = pools["bcast"].tile([P, 512], BF16, tag="rs_b", name="rs_b",
                                   bufs=2)
        nc.gpsimd.partition_broadcast(mu_b, st["mu2"][0:1, ch, :], channels=P)
        nc.gpsimd.partition_broadcast(rs_b, st["rs2"][0:1, ch, :], channels=P)
        for ct in range(CT):
            eng = nc.gpsimd if ct in pool_cts else nc.vector
            tmp = pools["tmp"].tile([P, 512], BF16, tag="tmp", name="tmp",
                                    bufs=2)
            eng.tensor_tensor(tmp, src_bf[:, ct, sl], mu_b, ALU.subtract)
            eng.tensor_tensor(tmp, tmp, rs_b, ALU.mult)
            out_writer(ct, sl, tmp)


def _emit_ln1_writes(tc, pools, pfx, io, mode="both"):
    """LN1 output writer: s8 (fp8, FFN1 input) and/or s16 (bf16 residual).

    mode "s8_pool" writes only s8, on the Pool engine (gelu-era DVE is
    saturated); the s16 pass is then recomputed later via mode "s16".
    """
    nc = tc.nc
    params = io["params"]
    if "s8" not in io:
        io["s8"] = pools["s_pool"].tile([P, CT, HW], FP8, tag=f"s8_{pfx}",
                                        name=f"s8_{pfx}")
        io["s16"] = pools["s_pool"].tile([P, CT, HW], BF16, tag=f"s16_{pfx}",
                                         name=f"s16_{pfx}")
    s8, s16 = io["s8"], io["s16"]

    def _ln1_write(ct, sl, tmp):
        if mode == "s16_to_pool":
            # s8 stays on DVE (critical path to the first gelus); s16
            # rides on Pool so av(7)-normalize starts earlier on DVE.
            nc.vector.tensor_scalar(
                out=s8[:, ct, sl], in0=tmp,
                scalar1=params["n1w"][:, ct:ct + 1],
                scalar2=params["n1b"][:, ct:ct + 1],
                op0=ALU.mult, op1=ALU.add,
            )
            nc.gpsimd.tensor_scalar(
                out=s16[:, ct, sl], in0=tmp,
                scalar1=params["n1w16"][:, ct:ct + 1],
                scalar2=params["n1b16"][:, ct:ct + 1],
                op0=ALU.mult, op1=ALU.add,
            )
            return
        if mode in ("both", "s8"):
            nc.vector.tensor_scalar(
                out=s8[:, ct, sl], in0=tmp,
                scalar1=params["n1w"][:, ct:ct + 1],
                scalar2=params["n1b"][:, ct:ct + 1],
                op0=ALU.mult, op1=ALU.add,
            )
        elif mode == "s8_pool":
            nc.gpsimd.tensor_scalar(
                out=s8[:, ct, sl], in0=tmp,
                scalar1=params["n1w"][:, ct:ct + 1],
                scalar2=params["n1b"][:, ct:ct + 1],
                op0=ALU.mult, op1=ALU.add,
            )
        if mode in ("both", "s16"):
            nc.vector.tensor_scalar(
                out=s16[:, ct, sl], in0=tmp,
                scalar1=params["n1w16"][:, ct:ct + 1],
                scalar2=params["n1b16"][:, ct:ct + 1],
                op0=ALU.mult, op1=ALU.add,
            )

    return _ln1_write


def _emit_ffn1(tc, pools, pfx, io, hts, after_ht=None):
    """FFN1 (fp8 DR, W1 x16) + gelu(scale=1/16) -> h fp8 [P, HT, HW].

    after_ht(ht) lets the caller drip other PE work (e.g. streamed FFN2
    accumulation over the just-finished h tiles) into the emission.
    """
    nc = tc.nc
    params = io["params"]
    w1 = io["w1"]
    if "h" not in io:
        io["h"] = pools["hbuf"].tile([P, HT, HW], FP8, tag="hbuf",
                                     name="hbuf")
    h = io["h"]
    s8 = io["s8"]
    for ht in hts:
        ph = pools["psum_s"].tile([P, HW], F32, tag="s", name="s")
        for ch in range(NCH):
            sl = slice(ch * 512, (ch + 1) * 512)
            for k in range(CT // 2):
                nc.tensor.matmul(
                    ph[:, sl],
                    lhsT=w1[:, 2 * k:2 * k + 2, ht * P:(ht + 1) * P],
                    rhs=s8[:, 2 * k:2 * k + 2, sl],
                    start=(k == 0), stop=(k == CT // 2 - 1),
                    perf_mode=DR,
                )
        nc.scalar.activation(
            out=h[:, ht, :], in_=ph, func=AF.Gelu,
            bias=params["b1"][:, ht:ht + 1], scale=1.0 / 16.0,
        )
        if after_ht is not None:
            after_ht(ht)


def _emit_ffn2_stream_start(tc, pools, io):
    """Held FFN2 ch0 accumulators for ct0/ct1 in the (idle) psum_av banks."""
    io["pfs"] = {
        ct: pools["psum_av"].tile([P, 512], F32, tag="av", name=f"pf{ct}")
        for ct in (0, 1)
    }


def _emit_ffn2_stream_k(tc, pools, io, k):
    """One k-pair of streamed FFN2 ch0 accumulation (needs h 2k,2k+1)."""
    nc = tc.nc
    w2, h = io["w2"], io["h"]
    for ct in (0, 1):
        nc.tensor.matmul(
            io["pfs"][ct],
            lhsT=w2[:, 2 * k:2 * k + 2, ct * P:(ct + 1) * P],
            rhs=h[:, 2 * k:2 * k + 2, 0:512],
            start=(k == 0), stop=(k == HT // 2 - 1),
            perf_mode=DR,
        )


def _emit_ffn2_stream_finish(tc, pools, pfx, io):
    """Evacuate the streamed ct0/ct1 ch0 psums -> r2 rows."""
    nc = tc.nc
    params = io["params"]
    if "r2" not in io:
        io["r2"] = pools["r_pool"].tile([P, CT, HW], BF16, tag=f"r_{pfx}",
                                        name=f"r2_{pfx}")
    for ct in (0, 1):
        nc.vector.scalar_tensor_tensor(
            out=io["r2"][:, ct, 0:512], in0=io["pfs"][ct],
            scalar=params["b216"][:, ct:ct + 1],
            in1=io["s16"][:, ct, 0:512], op0=ALU.add, op1=ALU.add,
        )
    del io["pfs"]


def _emit_ffn2(tc, pools, pfx, io, ct_chs):
    """FFN2 (fp8 DR, W2 x16) + 16*b2 + s16 residual -> r2_bf (=16*r2)."""
    nc = tc.nc
    params = io["params"]
    w2 = io["w2"]
    h = io["h"]
    psum_mm = pools["psum_mm"]
    if "r2" not in io:
        io["r2"] = pools["r_pool"].tile([P, CT, HW], BF16, tag=f"r_{pfx}",
                                        name=f"r2_{pfx}")
    r2_bf = io["r2"]
    for ct, ch in ct_chs:
        sl = slice(ch * 512, (ch + 1) * 512)
        pf = psum_mm.tile([P, 512], F32, tag="mm", name="mm")
        for k in range(HT // 2):
            nc.tensor.matmul(
                pf,
                lhsT=w2[:, 2 * k:2 * k + 2, ct * P:(ct + 1) * P],
                rhs=h[:, 2 * k:2 * k + 2, sl],
                start=(k == 0), stop=(k == HT // 2 - 1),
                perf_mode=DR,
            )
        nc.vector.scalar_tensor_tensor(
            out=r2_bf[:, ct, sl], in0=pf, scalar=params["b216"][:, ct:ct + 1],
            in1=io["s16"][:, ct, sl], op0=ALU.add, op1=ALU.add,
        )


def _emit_ln2_norm(tc, pools, pfx, io, ch, act_scale=False):
    """LN2 normalize+scale+DMA out for one chunk.

    act_scale=True puts the *w+b step on ACT (Identity, table-free) --
    only for tail chunks where ACT is otherwise idle.
    """
    nc = tc.nc
    params, out_dram = io["params"], io["out"]
    st = io["ln2st"]

    def _ln2_write(ct, sl, tmp):
        o32 = pools["ostage"].tile([P, 512], F32, tag="ostage", name="ostage",
                                   bufs=2)
        if act_scale:
            nc.scalar.activation(
                out=o32, in_=tmp, func=AF.Identity,
                bias=params["n2b"][:, ct:ct + 1],
                scale=params["n2w"][:, ct:ct + 1],
            )
        else:
            nc.vector.tensor_scalar(
                out=o32, in0=tmp,
                scalar1=params["n2w"][:, ct:ct + 1],
                scalar2=params["n2b"][:, ct:ct + 1],
                op0=ALU.mult, op1=ALU.add,
            )
        eng = (nc.sync, nc.gpsimd, nc.scalar, nc.sync)[ct % 4]
        eng.dma_start(out=out_dram[ct * P:(ct + 1) * P, sl], in_=o32)

    _emit_ln_norm(tc, pools, io["r2"], st, _ln2_write, chunks=(ch,))


def _emit_ln2_stats(tc, pools, pfx, io, ch, sq_dve=True, rsqrt=True):
    st = io.setdefault("ln2st", {})
    _emit_ln_stats(tc, pools, io["r2"], st, io["inv512"], io["eps"],
                   chunks=(ch,), sq_dve=sq_dve, rsqrt=rsqrt)


def build_program():
    nc = bacc.Bacc("TRN2", target_bir_lowering=False, debug=False)

    def din(name, shape, dt):
        return nc.dram_tensor(name, list(shape), dt, kind="ExternalInput").ap()

    x8 = {p: din(f"x_{p}8", (P, CT, HW), FP8) for p in "sf"}
    x256 = {p: din(f"x_{p}256", (P, CT, HW), BF16) for p in "sf"}
    wq8 = {p: din(f"{p}_wq8", (P, CT, C), FP8) for p in "sf"}
    wk8 = {p: din(f"{p}_wk8", (P, CT, C), FP8) for p in "sf"}
    wv8 = {p: din(f"{p}_wv8", (P, CT, C), FP8) for p in "sf"}
    wo8 = {p: din(f"{p}_wo8", (P, CT, C), FP8) for p in "sf"}
    w18 = {p: din(f"{p}_w18", (P, CT, HID), FP8) for p in "sf"}
    w28 = {p: din(f"{p}_w28", (P, HT, C), FP8) for p in "sf"}
    prm_d = din("prm", (P, 2, NPCOL), F32)
    outs = {
        p: nc.dram_tensor(f"out_{p}", [C, HW], F32, kind="ExternalOutput").ap()
        for p in "sf"
    }

    with tile.TileContext(nc) as tc:
        from contextlib import ExitStack
        with ExitStack() as ctx:
            pools = {}

            def pool(name, bufs, space="SBUF", stack=None):
                pools[name] = (stack or ctx).enter_context(
                    tc.tile_pool(name=name, bufs=bufs, space=space)
                )
                return pools[name]

            # whole-program pools
            pool("psum_mm", 2, space="PSUM")
            pool("psum_s", 2, space="PSUM")
            pool("psum_av", 2, space="PSUM")
            pool("consts", 1)
            pool("params", 1)
            pool("rows", 1)
            pool("xr", 2)
            pool("bcast", 1)
            pool("tmp", 1)
            pool("sq", 2)
            pool("rz", 1)
            pool("pt", 3)
            pool("qkv", 1)
            pool("o_pool", 1)
            pool("wo_pool", 1)
            pool("r_pool", 1)
            pool("s_pool", 1)
            pool("ostage", 2)
            # x8 + wproj live only through the projections; their SBUF is
            # reused by x256/wffn/hbuf afterwards (LIFO close below).
            xw_stack = ctx.enter_context(ExitStack())
            pool("x8", 1, stack=xw_stack)
            pool("wproj", 2, stack=xw_stack)

            inv512 = pools["consts"].tile([P, 1], BF16)
            nc.vector.memset(inv512, 1.0 / C)
            eps_sb = pools["consts"].tile([1, 1], F32)
            nc.vector.memset(eps_sb, EPS)
            # Pin the ACT table set to natural_log_exp_and_others (Ln+Exp)
            # before the softmax exps start.
            lnpin = pools["consts"].tile([1, 1], F32)
            nc.vector.memset(lnpin, 1.0)
            nc.scalar.activation(lnpin, lnpin, AF.Ln, bias=eps_sb[:, 0:1])

            # PE warm-up: junk matmuls spend the p-state ramp (~3us to full
            # clock) while the first DMAs land, so the real projections run
            # at full speed. Nothing reads the psum tiles.
            warm = pools["consts"].tile([P, 2, 512], FP8)
            nc.vector.memset(warm, 0.0625)
            for _ in range(22):
                pw = pools["psum_mm"].tile([P, 512], F32, tag="mm",
                                           name="warm")
                nc.tensor.matmul(pw, lhsT=warm[:, :, 0:P], rhs=warm,
                                 start=True, stop=True, perf_mode=DR)

            # ---- bulk loads: one contiguous DMA each, rotating queues ----
            _q = [nc.sync, nc.gpsimd, nc.scalar]
            _qi = [0]

            def dma_next(out, in_):
                eng = _q[_qi[0] % len(_q)]
                _qi[0] += 1
                eng.dma_start(out=out, in_=in_)

            x8_sb = {}
            w_sb = {}
            for p in "sf":
                x8_sb[p] = pools["x8"].tile([P, CT, HW], FP8, tag=f"x8_{p}",
                                            name=f"x8_{p}")
            for nm, src in (("wq_s", wq8["s"]), ("wk_s", wk8["s"])):
                w_sb[nm] = pools["wproj"].tile([P, CT, C], FP8, tag=nm[:2],
                                               name=nm)
            # head-critical loads first: x_s, wq_s, x_f, wk_s
            dma_next(x8_sb["s"], x8["s"])
            dma_next(w_sb["wq_s"], wq8["s"])
            dma_next(x8_sb["f"], x8["f"])
            dma_next(w_sb["wk_s"], wk8["s"])

            # params block (single small DMA)
            prm_sb = pools["params"].tile([P, 2, NPCOL], F32, tag="prm")
            dma_next(prm_sb, prm_d)
            params = {}
            for pi, p in enumerate("sf"):
                params[p] = {}
                for i, n in enumerate(PNAMES):
                    params[p][n] = prm_sb[:, pi, i * CT:(i + 1) * CT]
                params[p]["b1"] = prm_sb[:, pi, len(PNAMES) * CT:NPCOL]

            qkv = {}
            for p in "sf":
                for g in range(2):
                    qkv[f"q_{p}{g}"] = pools["qkv"].tile(
                        [P, 2, HW], FP8, tag=f"q_{p}{g}", name=f"q_{p}{g}")
                    qkv[f"k_{p}{g}"] = pools["qkv"].tile(
                        [P, 2, HW], FP8, tag=f"k_{p}{g}", name=f"k_{p}{g}")
                qkv[f"v_{p}"] = pools["qkv"].tile(
                    [P, TT, HEADS, VW], FP8, tag=f"v_{p}", name=f"v_{p}")

            wo_sb = {}
            o_sb = {}
            for p in "sf":
                wo_sb[p] = pools["wo_pool"].tile([P, CT, C], FP8,
                                                 tag=f"wo_{p}", name=f"wo_{p}")
                o_sb[p] = pools["o_pool"].tile([P, HEADS // 2, HW], FP8,
                                               tag=f"o_{p}", name=f"o_{p}")

            ios = {}
            for p in "sf":
                ios[p] = {
                    "o": o_sb[p], "wo": wo_sb[p],
                    "params": params[p], "out": outs[p],
                    "inv512": inv512, "eps": eps_sb,
                }

            # software-pipelined attention: S^T+exp of pair N overlaps
            # AV of pair N-1 on PE, so PE never waits on the ACT exp chain.
            # stream 's': q from x_s, kv from x_f ; stream 'f': swapped
            seq = [("s", hp) for hp in range(4)] + [("f", hp) for hp in range(4)]
            pts = {}

            def st(i, filler=None):
                p, hp = seq[i]
                g = hp // 2
                pts[i] = _emit_st_exp(tc, pools, hp, qkv[f"q_{p}{g}"],
                                      qkv[f"k_{p}{g}"], filler=filler)

            def av(i):
                p, hp = seq[i]
                _emit_av(tc, pools, hp, pts.pop(i), qkv[f"v_{p}"], o_sb[p])

            qs2 = [qkv["q_s0"], qkv["q_s1"]]
            ks2 = [qkv["k_s0"], qkv["k_s1"]]
            qf2 = [qkv["q_f0"], qkv["q_f1"]]
            kf2 = [qkv["k_f0"], qkv["k_f1"]]

            # ---- A(s): group g0 first so st(0)/st(1) start early ----
            _emit_proj_qk_one(tc, pools, x8_sb["s"], w_sb["wq_s"], qs2,
                              ts=(0, 1), act_evict=True)
            _emit_proj_qk_one(tc, pools, x8_sb["f"], w_sb["wk_s"], ks2,
                              ts=(0, 1), act_evict=True)
            st(0)
            _emit_proj_qk_one(tc, pools, x8_sb["s"], w_sb["wq_s"], qs2,
                              ts=(2, 3))
            _emit_proj_qk_one(tc, pools, x8_sb["f"], w_sb["wk_s"], ks2,
                              ts=(2, 3))
            st(1)
            # next round of loads
            w_sb["wv_s"] = pools["wproj"].tile([P, CT, C], FP8, tag="wv",
                                               name="wv_s")
            w_sb["wq_f"] = pools["wproj"].tile([P, CT, C], FP8, tag="wq",
                                               name="wq_f")
            w_sb["wk_f"] = pools["wproj"].tile([P, CT, C], FP8, tag="wk",
                                               name="wk_f")
            w_sb["wv_f"] = pools["wproj"].tile([P, CT, C], FP8, tag="wv",
                                               name="wv_f")
            dma_next(w_sb["wv_s"], wv8["s"])
            dma_next(w_sb["wq_f"], wq8["f"])
            dma_next(w_sb["wk_f"], wk8["f"])
            nc.vector.memset(qkv["v_s"][:, :, :, DH + 1:], 0.0)
            _emit_proj_v(tc, pools, x8_sb["f"], w_sb["wv_s"], qkv["v_s"])
            av(0)
            st(2)
            dma_next(w_sb["wv_f"], wv8["f"])
            dma_next(wo_sb["s"], wo8["s"])
            dma_next(wo_sb["f"], wo8["f"])
            av(1)
            _emit_proj_qk_one(tc, pools, x8_sb["f"], w_sb["wq_f"], qf2,
                              ts=(0, 1))
            _emit_proj_qk_one(tc, pools, x8_sb["s"], w_sb["wk_f"], kf2,
                              ts=(0, 1))
            st(3)
            av(2)
            _emit_proj_qk_one(tc, pools, x8_sb["f"], w_sb["wq_f"], qf2,
                              ts=(2, 3))
            _emit_proj_qk_one(tc, pools, x8_sb["s"], w_sb["wk_f"], kf2,
                              ts=(2, 3))
            nc.vector.memset(qkv["v_f"][:, :, :, DH + 1:], 0.0)
            _emit_proj_v(tc, pools, x8_sb["s"], w_sb["wv_f"], qkv["v_f"])
            # projections done: free x8/wproj SBUF for wffn/hbuf
            xw_stack.close()
            pool("wffn", 1)
            pool("hbuf", 1)
            for p in "sf":
                ios[p]["x256d"] = x256[p]
            ios["s"]["w1"] = pools["wffn"].tile([P, CT, HID], FP8, tag="w1",
                                                name="w1_s", bufs=1)
            ios["s"]["w2"] = pools["wffn"].tile([P, HT, C], FP8, tag="w2",
                                                name="w2_s", bufs=2)
            dma_next(ios["s"]["w1"], w18["s"])
            dma_next(ios["s"]["w2"], w28["s"])
            # av/wo/LN1(s) ride as fillers INSIDE the st blocks: their PE
            # work lands in the PE slack of the block instead of delaying
            # the next block's S^T matmuls (which would starve the exps).
            st(4)
            av(3)
            st(5)
            _emit_wo_residual(tc, pools, "s", ios["s"], (0, 1))
            av(4)
            st(6)
            _emit_wo_residual(tc, pools, "s", ios["s"], (2, 3))
            av(5)
            # LN1(s) fully inside the attention era: rsqrt shares the
            # softmax Ln+Exp table set, the normalize rides the idle DVE
            # slack, so s8 is ready the moment the last exp retires.
            ios["s"]["ln1st"] = {}
            _emit_ln_stats(tc, pools, ios["s"]["r"], ios["s"]["ln1st"],
                           inv512, eps_sb, rsqrt=True)
            st(7)
            av(6)
            _emit_ln_norm(tc, pools, ios["s"]["r"], ios["s"]["ln1st"],
                          _emit_ln1_writes(tc, pools, "s", ios["s"]))
            av(7)

            # ---- gelu era. ACT order: gelus(s), ln1(f), gelus(f),
            # ln2(s,0), ln2(s,1), ln2(f,0), ln2(f,1) -> 4 table reloads.
            # FFN2 ch0 ct0/ct1 accumulate in the idle psum_av banks DURING
            # the gelus (k-pair k right after gelu 2k+1), so only ct2/ct3
            # remain after the last gelu. gelus(s) start right after the
            # last exp; wo_residual(f) runs on PE behind the FFN1(s) mms.
            # DVE is the scarce engine during gelus(s): LN1(f) writes only
            # s8 there (TSPs on Pool); the s16 pass is recomputed during
            # gelus(f) when DVE is light again.
            _emit_ffn2_stream_start(tc, pools, ios["s"])
            ios["f"]["ln1st"] = {}

            def s_after_ht(ht):
                # Only drip PE work whose deps are certainly met at this
                # queue position (PE is in-order: a stalled drip matmul
                # blocks the FFN1 matmuls and starves the gelus).
                if ht % 2 == 1:
                    _emit_ffn2_stream_k(tc, pools, ios["s"], ht // 2)

            _emit_ffn1(tc, pools, "s", ios["s"], range(0, 16),
                       after_ht=s_after_ht)
            _emit_wo_residual(tc, pools, "f", ios["f"], (0, 1))
            _emit_wo_residual(tc, pools, "f", ios["f"], (2, 3))
            _emit_ln_stats(tc, pools, ios["f"]["r"], ios["f"]["ln1st"],
                           inv512, eps_sb, rsqrt=True)
            _emit_ln_norm(tc, pools, ios["f"]["r"], ios["f"]["ln1st"],
                          _emit_ln1_writes(tc, pools, "f", ios["f"],
                                           mode="s8_pool"))
            _emit_ffn2_stream_finish(tc, pools, "s", ios["s"])
            _emit_ffn2(tc, pools, "s", ios["s"], [(2, 0), (3, 0)])
            _emit_ln2_stats(tc, pools, "s", ios["s"], 0, rsqrt=False)
            ios["f"]["w1"] = pools["wffn"].tile([P, CT, HID], FP8, tag="w1",
                                                name="w1_f", bufs=1)
            dma_next(ios["f"]["w1"], w18["f"])
            ios["f"]["w2"] = pools["wffn"].tile([P, HT, C], FP8, tag="w2",
                                                name="w2_f", bufs=2)
            dma_next(ios["f"]["w2"], w28["f"])
            # f-stream gelus; FFN2(f) ch0 streams into psum_av, and the
            # four FFN2(s) ch1 column blocks drip in behind h_s (complete).
            _emit_ffn2_stream_start(tc, pools, ios["f"])
            # s16(f) is only needed by the FFN2(f) stream-finish stts;
            # recomputing it here (gelus(f) window) keeps it off the
            # saturated gelus(s)-era DVE.
            _emit_ln_norm(tc, pools, ios["f"]["r"], ios["f"]["ln1st"],
                          _emit_ln1_writes(tc, pools, "f", ios["f"],
                                           mode="s16"))

            def f_after_ht(ht):
                if ht % 2 == 1:
                    _emit_ffn2_stream_k(tc, pools, ios["f"], ht // 2)
                    if ht <= 7:
                        _emit_ffn2(tc, pools, "s", ios["s"],
                                   [((ht - 1) // 2, 1)])

            _emit_ffn1(tc, pools, "f", ios["f"], range(0, 16),
                       after_ht=f_after_ht)
            _emit_ln_rsqrt(tc, pools, ios["s"]["ln2st"], eps_sb, chunks=(0,))
            # ln2(s,1) stats go ahead of ln2(s,0)-normalize on DVE: the
            # normalize only feeds the output DMA, the stats gate an ACT op.
            _emit_ln2_stats(tc, pools, "s", ios["s"], 1)
            _emit_ln2_norm(tc, pools, "s", ios["s"], 0)
            _emit_ln2_norm(tc, pools, "s", ios["s"], 1)
            _emit_ffn2_stream_finish(tc, pools, "f", ios["f"])
            _emit_ffn2(tc, pools, "f", ios["f"], [(2, 0), (3, 0)])
            _emit_ln2_stats(tc, pools, "f", ios["f"], 0)
            _emit_ffn2(tc, pools, "f", ios["f"],
                       [(ct, 1) for ct in range(CT)])
            _emit_ln2_stats(tc, pools, "f", ios["f"], 1)
            _emit_ln2_norm(tc, pools, "f", ios["f"], 0, act_scale=True)
            _emit_ln2_norm(tc, pools, "f", ios["f"], 1, act_scale=True)

    nc.compile()
    return nc


# --------------------------------------------------------------------------
# host side
# --------------------------------------------------------------------------

_BF = ml_dtypes.bfloat16
_F8 = ml_dtypes.float8_e4m3
WS = 16.0  # host weight scale for fp8 matmuls
XS = WS * WS  # attention product scale absorbed by LN1


def _head_perm():
    """Permuted output-channel order for Q/K projections.

    Tile t = 2g+hi, partition 32*h4+lo  ->  orig channel (4g+h4)*64+32*hi+lo.
    """
    perm = np.zeros(C, dtype=np.int64)
    i = 0
    for g in range(2):
        for hi in range(2):
            for h4 in range(4):
                for lo in range(32):
                    perm[i] = (4 * g + h4) * 64 + 32 * hi + lo
                    i += 1
    return perm


def _pmajor(m, tiles):
    """[tiles*P, X] -> [P, tiles, X] with row c = t*P + p."""
    return np.ascontiguousarray(
        m.reshape(tiles, P, m.shape[1]).transpose(1, 0, 2)
    )


def _prep_shared_inputs(inputs):
    """Host-side weight prep: transposes, casts, permutations, x16 scales."""
    sh = {}
    perm = _head_perm()
    prm = np.zeros((P, 2, NPCOL), np.float32)
    for pi, (p, ap) in enumerate((("s", "s_"), ("f", "f_"))):
        wq, wk, wv, wo = (inputs[ap + n] for n in ("Wq", "Wk", "Wv", "Wo"))
        sh[f"{p}_wq8"] = _pmajor((wq.T[:, perm] * WS).astype(_F8), CT)
        sh[f"{p}_wk8"] = _pmajor((wk.T[:, perm] * WS).astype(_F8), CT)
        sh[f"{p}_wv8"] = _pmajor((wv.T * WS).astype(_F8), CT)
        sh[f"{p}_wo8"] = _pmajor((wo.T * WS).astype(_F8), CT)
        w1 = inputs[f"{p}ffn_W1"]
        w2 = inputs[f"{p}ffn_W2"]
        sh[f"{p}_w18"] = _pmajor((w1.T * WS).astype(_F8), CT)
        sh[f"{p}_w28"] = _pmajor((w2.T * WS).astype(_F8), HT)
        vals = {
            "bo256": inputs[ap + "bo"] * XS,
            "n1w": inputs[f"{p}n1_w"], "n1b": inputs[f"{p}n1_b"],
            "n1w16": inputs[f"{p}n1_w"] * WS,
            "n1b16": inputs[f"{p}n1_b"] * WS,
            "n2w": inputs[f"{p}n2_w"], "n2b": inputs[f"{p}n2_b"],
            "b216": inputs[f"{p}ffn_b2"] * WS,
        }
        for i, n in enumerate(PNAMES):
            prm[:, pi, i * CT:(i + 1) * CT] = vals[n].reshape(CT, P).T
        prm[:, pi, len(PNAMES) * CT:NPCOL] = (
            inputs[f"{p}ffn_b1"].reshape(HT, P).T
        )
    sh["prm"] = prm
    return sh


def make_in_maps(inputs):
    shared = _prep_shared_inputs(inputs)
    xs = np.ascontiguousarray(inputs["spatial_feat"].reshape(B, C, HW))
    xf = np.ascontiguousarray(inputs["freq_feat"].reshape(B, C, HW))
    in_maps = []
    for b in range(N_CORES):
        m = dict(shared)
        m["x_s8"] = _pmajor(xs[b].astype(_F8), CT)
        m["x_f8"] = _pmajor(xf[b].astype(_F8), CT)
        m["x_s256"] = _pmajor((xs[b] * XS).astype(_BF), CT)
        m["x_f256"] = _pmajor((xf[b] * XS).astype(_BF), CT)
        in_maps.append(m)
    return in_maps


_CACHED = {}


def _get_program():
    if "nc" not in _CACHED:
        _CACHED["nc"] = build_program()
    return _CACHED["nc"]


def run_on_hw(inputs, trace=False, trace_kwargs=None):
    from concourse.bass_utils import run_bass_kernel_spmd

    nc = _get_program()
    in_maps = make_in_maps(inputs)
    res = run_bass_kernel_spmd(
        nc, in_maps, list(range(N_CORES)), trace=trace,
        **(dict(trace_kwargs=trace_kwargs) if trace_kwargs else {}),
    )
    s = np.stack([res.results[b]["out_s"] for b in range(B)])
    f = np.stack([res.results[b]["out_f"] for b in range(B)])
    s = s.reshape(B, C, H_IMG, W_IMG).astype(np.float32)
    f = f.reshape(B, C, H_IMG, W_IMG).astype(np.float32)
    return (s, f), res


def kernel(**inputs):
    out, _ = run_on_hw(inputs, trace=False)
    return out


if __name__ == "__main__":
    import reference

    inputs = {k: np.asarray(v) for k, v in reference.setup_inputs().items()}
    exp_s, exp_f = reference.reference(**inputs)
    act_s, act_f = kernel(**inputs)
    for nm, e, a in (("s", exp_s, act_s), ("f", exp_f, act_f)):
        err = np.abs(np.asarray(a) - np.asarray(e)).max()
        print(nm, "absmax", err, "rel", err / np.abs(e).max())
